# revision 1
# baseline (speedup 1.0000x reference)
"""Gated spiking reservoir step — Trainium2 Bass kernel (8 NeuronCores).

Math (per reference):
    ic   = inputs @ input_weights                  # [B, R]
    rc   = reservoir_state @ reservoir_weights     # [B, R]
    gate = sigmoid(inputs @ gate_weights)          # [B, R]
    ns   = (0.9 * reservoir_state + 0.1 * tanh(ic + rc)) * gate
    out  = (ns > 0.5) ? 1.0 : 0.0
    returns (out, ns)

Sharding: tensor-parallel over the reservoir (output-column) dim.  Each of
the 8 cores owns a 512-column slice of all three weight matrices and
produces the matching [512-column x full-batch] slice of both outputs.
The activations (inputs / reservoir_state) are replicated, pre-transposed
on host to [K, B] so the contraction dim lands on SBUF partitions.

On-device layout per core (everything transposed — state tiles are
[cols(part) x batch(free)]):
    for each batch slice of 512:
        gate_ps[c]  += w_gate[k-tile, c-tile].T @ xT[k-tile, b-slice]   (8 k-tiles)
        state_ps[c] += w_in  [k-tile, c-tile].T @ xT[k-tile, b-slice]   (8 k-tiles)
        state_ps[c] += w_res [k-tile, c-tile].T @ sT[k-tile, b-slice]   (32 k-tiles)
        t  = tanh(state_ps[c])        (ScalarE, from PSUM)
        g  = sigmoid(gate_ps[c])      (ScalarE, from PSUM)
        v  = (s_slice * 9.0) + t      (VectorE scalar_tensor_tensor)
        ns = (v * 0.1) * g            (VectorE scalar_tensor_tensor)
        spk = ns > 0.5                (VectorE tensor_scalar is_gt -> 1.0/0.0)
"""

import os
import sys

if "/opt/trn_rl_repo" not in sys.path:
    sys.path.insert(0, "/opt/trn_rl_repo")

import numpy as np

B, D_IN, R = 2048, 1024, 4096
N_CORES = 8
COLS = R // N_CORES          # 512 output columns per core
P = 128                      # SBUF/PSUM partitions
NB = 512                     # batch free-dim per matmul / PSUM bank
KI = D_IN // P               # 8 k-tiles over the input dim
KR = R // P                  # 32 k-tiles over the reservoir dim
CT = COLS // P               # 4 column tiles per core
BT = B // NB                 # 4 batch slices

# float32r runs the PE at 4x the fp32 rate for moving dim >= 256.
MM_DTYPE = os.environ.get("BASS_MM_DTYPE", "float32r")

_CACHE = {}


def _build(mm_dtype_name: str):
    from contextlib import ExitStack

    from concourse import bacc, tile
    import concourse.mybir as mybir

    f32 = mybir.dt.float32
    mm_dt = getattr(mybir.dt, mm_dtype_name)
    AF = mybir.ActivationFunctionType
    ALU = mybir.AluOpType

    nc = bacc.Bacc(
        "TRN2", target_bir_lowering=False, debug=False, enable_asserts=False
    )

    xT = nc.dram_tensor("xT", [D_IN, B], f32, kind="ExternalInput")
    sT = nc.dram_tensor("sT", [R, B], f32, kind="ExternalInput")
    w_in = nc.dram_tensor("w_in", [D_IN, COLS], f32, kind="ExternalInput")
    w_res = nc.dram_tensor("w_res", [R, COLS], f32, kind="ExternalInput")
    w_gate = nc.dram_tensor("w_gate", [D_IN, COLS], f32, kind="ExternalInput")
    nsT = nc.dram_tensor("nsT", [COLS, B], f32, kind="ExternalOutput")
    spkT = nc.dram_tensor("spkT", [COLS, B], mybir.dt.uint8, kind="ExternalOutput")

    def cast(ap):
        return ap.bitcast(mm_dt) if mm_dtype_name != "float32" else ap

    with tile.TileContext(nc) as tc, ExitStack() as ctx:
        # Resident weights: 12 MB of SBUF (96 KB/partition), one tile per
        # 128-row k-slice so matmuls only wait on the slice they consume.
        wpool = ctx.enter_context(tc.tile_pool(name="weights", bufs=1))
        w_in_sb, w_gate_sb, w_res_sb = [], [], []
        for k in range(KI):
            t = wpool.tile([P, COLS], mm_dt, tag=f"w_in_{k}", name=f"w_in_sb{k}")
            w_in_sb.append(t)
            t = wpool.tile([P, COLS], mm_dt, tag=f"w_gate_{k}", name=f"w_gate_sb{k}")
            w_gate_sb.append(t)
        for k in range(KR):
            t = wpool.tile([P, COLS], mm_dt, tag=f"w_res_{k}", name=f"w_res_sb{k}")
            w_res_sb.append(t)

        # All weights are JIT-issued on HWDGE right before first use
        # (w_in/w_gate in slice 0's x-phase, w_res in slice 0's s-phase).
        def load_w_res(k):
            nc.sync.dma_start(w_res_sb[k][:], cast(w_res[k * P : (k + 1) * P, :]))

        xpool = ctx.enter_context(tc.tile_pool(name="x_mov", bufs=6))
        spool = ctx.enter_context(tc.tile_pool(name="s_mov", bufs=6))
        st_psum = ctx.enter_context(tc.tile_pool(name="st_ps", bufs=4, space="PSUM"))
        gt_psum = ctx.enter_context(tc.tile_pool(name="gt_ps", bufs=4, space="PSUM"))
        epool = ctx.enter_context(tc.tile_pool(name="epilogue", bufs=3))

        for b in range(BT):
            bs = slice(b * NB, (b + 1) * NB)
            state_ps = [st_psum.tile([P, NB], f32, tag="state", name=f"state_ps_{b}_{i}") for i in range(CT)]
            gate_ps = [gt_psum.tile([P, NB], f32, tag="gate", name=f"gate_ps_{b}_{i}") for i in range(CT)]

            # Gate matmuls first so gate PSUM banks retire early.
            for k in range(KI):
                xt = xpool.tile([P, NB], mm_dt, tag="xt")
                nc.sync.dma_start(xt[:], cast(xT[k * P : (k + 1) * P, bs]))
                if b == 0:
                    nc.sync.dma_start(
                        w_gate_sb[k][:], cast(w_gate[k * P : (k + 1) * P, :])
                    )
                    nc.sync.dma_start(
                        w_in_sb[k][:], cast(w_in[k * P : (k + 1) * P, :])
                    )
                for c in range(CT):
                    nc.tensor.matmul(
                        gate_ps[c][:],
                        w_gate_sb[k][:, c * P : (c + 1) * P],
                        xt[:],
                        start=(k == 0),
                        stop=(k == KI - 1),
                    )
                for c in range(CT):
                    nc.tensor.matmul(
                        state_ps[c][:],
                        w_in_sb[k][:, c * P : (c + 1) * P],
                        xt[:],
                        start=(k == 0),
                        stop=False,
                    )
            # s-phase A: first half of the k-tiles, all column tiles in
            # lockstep (k-major) so each st tile is short-lived.
            KH = KR // 2
            st_ep = []
            for k in range(KH):
                if b == 0:
                    load_w_res(k)
                if k < CT:
                    # This core's own state rows (epilogue reads them too):
                    # keep an exact fp32 copy, round to f32r on-chip for PE.
                    sf = spool.tile([P, NB], f32, tag="stEp", bufs=9,
                                    name=f"stEp_{b}_{k}")
                    nc.sync.dma_start(sf[:], sT[k * P : (k + 1) * P, bs])
                    st_ep.append(sf)
                    if mm_dtype_name != "float32":
                        st = spool.tile([P, NB], mm_dt, tag="stEpR", bufs=3,
                                        name=f"stEpR_{b}_{k}")
                        nc.scalar.copy(st[:], sf[:])
                    else:
                        st = sf
                else:
                    st = spool.tile([P, NB], mm_dt, tag="st")
                    nc.sync.dma_start(st[:], cast(sT[k * P : (k + 1) * P, bs]))
                for c in range(CT):
                    nc.tensor.matmul(
                        state_ps[c][:],
                        w_res_sb[k][:, c * P : (c + 1) * P],
                        st[:],
                        start=False,
                        stop=False,
                    )
            # s-phase B: second half column-major, so state_ps[c] finishes
            # (and its PSUM slot frees via tanh) staggered well before the
            # slice ends -- removes the PE bubble at slice boundaries.
            stB = []
            for k in range(KH, KR):
                if b == 0:
                    load_w_res(k)
                st = spool.tile([P, NB], mm_dt, tag="stB", bufs=18, name=f"stB_{b}_{k}")
                nc.sync.dma_start(st[:], cast(sT[k * P : (k + 1) * P, bs]))
                stB.append(st)
            for c in range(CT):
                for j, k in enumerate(range(KH, KR)):
                    nc.tensor.matmul(
                        state_ps[c][:],
                        w_res_sb[k][:, c * P : (c + 1) * P],
                        stB[j][:],
                        start=False,
                        stop=(k == KR - 1),
                    )

            NH = NB // 2
            for c in range(CT):
                cs = slice(c * P, (c + 1) * P)
                se_f32 = st_ep[c]
                ns = epool.tile([P, NB], f32, tag="ns", name=f"ns_{b}_{c}")
                spk = epool.tile([P, NB], mybir.dt.uint8, tag="spk",
                                 name=f"spk_{b}_{c}")
                for h in range(2):
                    hs = slice(h * NH, (h + 1) * NH)
                    tt = epool.tile([P, NH], f32, tag="tanh")
                    nc.scalar.activation(tt[:], state_ps[c][:, hs], AF.Tanh)
                    gg = epool.tile([P, NH], f32, tag="sig")
                    nc.scalar.activation(gg[:], gate_ps[c][:, hs], AF.Sigmoid)
                    vv = epool.tile([P, NH], f32, tag="v")
                    nc.vector.scalar_tensor_tensor(
                        vv[:], se_f32[:, hs], 9.0, tt[:], ALU.mult, ALU.add
                    )
                    nc.vector.scalar_tensor_tensor(
                        ns[:, hs], vv[:], 0.1, gg[:], ALU.mult, ALU.mult
                    )
                    nc.vector.tensor_scalar(
                        spk[:, hs], ns[:, hs], 0.5, None, ALU.is_gt
                    )
                nc.sync.dma_start(nsT[cs, bs], ns[:])
                nc.sync.dma_start(spkT[cs, bs], spk[:])

    nc.compile()
    return nc


def _get_program():
    if MM_DTYPE not in _CACHE:
        _CACHE[MM_DTYPE] = _build(MM_DTYPE)
    return _CACHE[MM_DTYPE]


def kernel(inputs, prev_output, reservoir_state, input_weights, reservoir_weights,
           gate_weights):
    from concourse.bass_utils import run_bass_kernel_spmd

    nc = _get_program()

    x = np.ascontiguousarray(np.asarray(inputs, dtype=np.float32))
    s = np.ascontiguousarray(np.asarray(reservoir_state, dtype=np.float32))
    w_in = np.ascontiguousarray(np.asarray(input_weights, dtype=np.float32))
    w_res = np.ascontiguousarray(np.asarray(reservoir_weights, dtype=np.float32))
    w_gate = np.ascontiguousarray(np.asarray(gate_weights, dtype=np.float32))

    xT = np.ascontiguousarray(x.T)          # [D_IN, B]
    sT = np.ascontiguousarray(s.T)          # [R, B]

    in_maps = []
    for core in range(N_CORES):
        c0 = core * COLS
        cs = slice(c0, c0 + COLS)
        # Rotate the contraction (reservoir-row) order so this core's own
        # 512 state rows arrive as k-tiles 0..3 -- the epilogue reuses those
        # SBUF tiles directly instead of re-reading them from HBM.  The same
        # rotation is applied to w_res rows, so the dot products are
        # unchanged (summation is commutative).
        w_res_c = w_res[:, cs]
        in_maps.append(
            {
                "xT": xT,
                "sT": np.concatenate([sT[c0:], sT[:c0]], axis=0),
                "w_in": np.ascontiguousarray(w_in[:, cs]),
                "w_res": np.concatenate([w_res_c[c0:], w_res_c[:c0]], axis=0),
                "w_gate": np.ascontiguousarray(w_gate[:, cs]),
            }
        )

    res = run_bass_kernel_spmd(nc, in_maps, list(range(N_CORES)))

    ns_T = np.concatenate([res.results[c]["nsT"] for c in range(N_CORES)], axis=0)
    spk_T = np.concatenate([res.results[c]["spkT"] for c in range(N_CORES)], axis=0)
    new_state = np.ascontiguousarray(ns_T.T)     # [B, R]
    output = spk_T.T.astype(np.float32)          # [B, R]

    # The PE's fast fp32 path (float32r) rounds matmul operands to ~12
    # mantissa bits, so new_state carries ~3e-4 absolute error.  That only
    # matters for the binary spike output where new_state sits within that
    # error of the 0.5 threshold.  Re-evaluate just those borderline
    # elements (~0.1% of the tensor) at full precision and patch both
    # outputs, restoring plain-fp32-level accuracy for the thresholding.
    bi, rj = np.nonzero(np.abs(new_state - 0.5) < 1.5e-3)
    if bi.size:
        xg = x[bi].astype(np.float64)
        sg = s[bi].astype(np.float64)
        acc = np.einsum("ij,ji->i", xg, w_in[:, rj], optimize=True)
        acc += np.einsum("ij,ji->i", sg, w_res[:, rj], optimize=True)
        gate = 1.0 / (1.0 + np.exp(-np.einsum("ij,ji->i", xg, w_gate[:, rj],
                                              optimize=True)))
        ns_fix = (0.9 * s[bi, rj].astype(np.float64)
                  + 0.1 * np.tanh(acc)) * gate
        ns_fix32 = ns_fix.astype(np.float32)
        new_state[bi, rj] = ns_fix32
        output[bi, rj] = (ns_fix32 > 0.5).astype(np.float32)
    return output, new_state



# revision 19
# speedup vs baseline: 3.3965x; 3.3965x over previous
"""Gated spiking reservoir step — Trainium2 Bass kernel (8 NeuronCores).

Math (per reference):
    ic   = inputs @ input_weights                  # [B, R]
    rc   = reservoir_state @ reservoir_weights     # [B, R]
    gate = sigmoid(inputs @ gate_weights)          # [B, R]
    ns   = (0.9 * reservoir_state + 0.1 * tanh(ic + rc)) * gate
    out  = (ns > 0.5) ? 1.0 : 0.0
    returns (out, ns)

Sharding: hybrid tensor/data parallel — 4 column groups x 2 batch halves.
Core (m, cg) owns a 1024-column slice of the three weight matrices and a
1024-row batch half, producing the [1024-col x 1024-batch] tile of both
outputs.  This minimizes per-core HBM traffic (~15 MB: weights 6 +
activations 5 + epilogue-state 2 + output 2).

All three GEMMs run in fp8 (e4m3) with the PE's DoubleRow perf mode: one
matmul instruction contracts 256 rows (2 fp8 rows per partition) at 0.5
cycles/row — 4x the fp32r rate.  Operands are pre-scaled on host (x*16,
s*16, w*512) so fp8 quantization lands in e4m3's normal range; the 1/8192
descale folds into the tanh/sigmoid activation scale.

The batch half is processed in 2 blocks of 512.  PSUM: 8 banks, one per
column tile of 128; within a block each bank first accumulates the gate
GEMM (group closed early, sigmoid drains it), then the state GEMM
(input + reservoir) under a second accumulation group.

Epilogue per column tile: sigmoid/tanh on ScalarE (from PSUM, descaled),
then fast 2x-mode fp16 VectorE ops: u = (0.9s)*g and g01 = 0.1*g are
issued right after the sigmoid (off the critical path; host pre-scales
sepi by 0.9), leaving only m = tanh*g01 and ns = u + m after the last
matmul.  fp16 transposed output DMAs per column tile.

Host side: assemble/transpose, spike threshold, and an exact float64
re-evaluation of elements within BAND of the 0.5 threshold (fp8 matmul
noise ~4e-3 RMS would otherwise flip spikes near the boundary).
"""

import sys

if "/opt/trn_rl_repo" not in sys.path:
    sys.path.insert(0, "/opt/trn_rl_repo")

import numpy as np

B, D_IN, R = 2048, 1024, 4096
N_CORES = 8
N_CG = 4                     # column groups
N_BG = 2                     # batch groups
COLS = R // N_CG             # 1024 output columns per core
B_CORE = B // N_BG           # 1024 batch rows per core
P = 128                      # SBUF/PSUM partitions
CT = COLS // P               # 8 column tiles per core
NB = 512                     # batch block (one PSUM bank of f32)
NH = NB // 2                 # moving free dim per DoubleRow matmul
NBLK = B_CORE // NB          # 2 batch blocks
QX = D_IN // 256             # 4 double-row contraction tiles over input dim
QS = R // 256                # 16 double-row contraction tiles over reservoir
SG = 4                       # s8/w_res chunk granularity: 4 q-tiles

SCALE_X = 16.0               # fp8 pre-scale for activations (x and s)
SCALE_W = 512.0              # fp8 pre-scale for weights
DESCALE = 1.0 / (SCALE_X * SCALE_W)

# Re-evaluate elements this close to the 0.5 spike threshold exactly on
# host (fp8 matmul noise sigma ~4e-3; 0.018 ~ 4.5 sigma).
BAND = 0.018

_CACHE = {}


def _build():
    from contextlib import ExitStack

    from concourse import bacc, tile
    import concourse.mybir as mybir

    f32 = mybir.dt.float32
    f16 = mybir.dt.float16
    fp8 = mybir.dt.float8e4
    AF = mybir.ActivationFunctionType
    ALU = mybir.AluOpType
    DR = mybir.MatmulPerfMode.DoubleRow

    nc = bacc.Bacc(
        "TRN2", target_bir_lowering=False, debug=False, enable_asserts=False
    )

    # DRAM layouts (host pre-packed so every DMA is contiguous >=512B/row):
    #   x8   [512, 2048]  row q*128+p, free nb*1024 + jj*512 + i
    #                     = fp8(16 * x.T)[q*256+jj*128+p, bg*1024 + nb*512+i]
    #   s8   [2048, 2048] same layout over the 4096 reservoir rows
    #   w_*8 [128, J*COLS] row p, free j*COLS + c = fp8(512*w)[j*128+p, c]
    #   sepi [1024, 1024] fp16 (0.9 * s.T)[cols, batch-half]
    #   ns   [1024, 1024] fp16 output, same [col, batch] layout
    x8 = nc.dram_tensor("x8", [4 * P, NBLK * 1024], fp8, kind="ExternalInput")
    s8 = nc.dram_tensor("s8", [16 * P, NBLK * 1024], fp8, kind="ExternalInput")
    w_in8 = nc.dram_tensor("w_in8", [P, 8 * COLS], fp8, kind="ExternalInput")
    w_res8 = nc.dram_tensor("w_res8", [P, 32 * COLS], fp8, kind="ExternalInput")
    w_gate8 = nc.dram_tensor("w_gate8", [P, 8 * COLS], fp8, kind="ExternalInput")
    sepi = nc.dram_tensor("sepi", [COLS, B_CORE], f16, kind="ExternalInput")
    ns_out = nc.dram_tensor("ns_out", [COLS, B_CORE], f16, kind="ExternalOutput")

    def wslice(t, j2, ct):
        # [128, 2, 128] stationary view: rows {j2, j2+1}, cols ct*128..+128
        return t[:, j2 * COLS : (j2 + 2) * COLS].rearrange(
            "p (j c) -> p j c", j=2
        )[:, :, ct * P : (ct + 1) * P]

    def mslice(t, q, h):
        # [128, 2, 256] moving view from a flat [128, q*1024] block tile
        return t[:, q * 1024 : (q + 1) * 1024].rearrange(
            "p (j i) -> p j i", j=2
        )[:, :, h * NH : (h + 1) * NH]

    with tile.TileContext(nc) as tc, ExitStack() as ctx:
        wpool = ctx.enter_context(tc.tile_pool(name="weights", bufs=1))
        wg_sb = wpool.tile([P, 8 * COLS], fp8, tag="wg", name="wg_sb")
        wi_sb = wpool.tile([P, 8 * COLS], fp8, tag="wi", name="wi_sb")
        wr_sb = [
            wpool.tile([P, 2 * SG * COLS], fp8, tag=f"wr{g}", name=f"wr_sb{g}")
            for g in range(QS // SG)
        ]

        xpool = ctx.enter_context(tc.tile_pool(name="x_mov", bufs=NBLK))
        spool = ctx.enter_context(tc.tile_pool(name="s_mov", bufs=QS // SG * NBLK))
        sepool = ctx.enter_context(tc.tile_pool(name="s_epi", bufs=NBLK))
        banks = ctx.enter_context(tc.tile_pool(name="banks", bufs=CT, space="PSUM"))
        ggpool = ctx.enter_context(tc.tile_pool(name="gg", bufs=2 * CT))
        g1pool = ctx.enter_context(tc.tile_pool(name="gg01", bufs=2 * CT))
        uupool = ctx.enter_context(tc.tile_pool(name="uu", bufs=2 * CT))
        ttpool = ctx.enter_context(tc.tile_pool(name="tt", bufs=CT))
        vvpool = ctx.enter_context(tc.tile_pool(name="vv", bufs=CT))
        nspool = ctx.enter_context(tc.tile_pool(name="ns", bufs=2 * CT))

        # Everything is SBUF-resident (~14 MB): issue ALL input DMAs upfront
        # in consumption order (the first matmuls need half of w_gate plus
        # block 0's x, so those two lead), and the SP queue never head-of-line
        # blocks a load behind an output store that waits on compute.
        xts, seps, s_tss = [], [], []

        def load_x(nb):
            xt = xpool.tile([P, QX * 1024], fp8, tag="x", name=f"x_{nb}")
            nc.sync.dma_start(
                xt[:].rearrange("p (q f) -> p q f", q=QX),
                x8[:, nb * 1024 : (nb + 1) * 1024].rearrange(
                    "(q p) f -> p q f", p=P
                ),
            )
            xts.append(xt)

        def load_wg_half(k):
            hw = COLS // 2
            nc.sync.dma_start(
                wg_sb[:].rearrange("p (j c) -> p j c", j=8)
                [:, :, k * hw : (k + 1) * hw],
                w_gate8[:, :].rearrange("p (j c) -> p j c", j=8)
                [:, :, k * hw : (k + 1) * hw],
            )

        load_wg_half(0)
        load_x(0)
        load_wg_half(1)
        load_x(1)
        nc.sync.dma_start(wi_sb[:], w_in8[:, :])
        for nb in range(NBLK):
            bs1k = slice(nb * 1024, (nb + 1) * 1024)
            bs = slice(nb * NB, (nb + 1) * NB)
            s_ts = []
            for g in range(QS // SG):
                if nb == 0:
                    nc.sync.dma_start(
                        wr_sb[g][:],
                        w_res8[:, g * 2 * SG * COLS : (g + 1) * 2 * SG * COLS],
                    )
                st = spool.tile([P, SG * 1024], fp8, tag="s8", name=f"s8_{nb}_{g}")
                nc.sync.dma_start(
                    st[:].rearrange("p (q f) -> p q f", q=SG),
                    s8[g * SG * P : (g + 1) * SG * P, bs1k].rearrange(
                        "(q p) f -> p q f", p=P
                    ),
                )
                s_ts.append(st)
            s_tss.append(s_ts)
            sep = sepool.tile([P, CT * NB], f16, tag="sepi", name=f"sepi_{nb}")
            nc.sync.dma_start(
                sep[:].rearrange("p (c f) -> p c f", c=CT),
                sepi[:, bs].rearrange("(c p) f -> p c f", p=P),
            )
            seps.append(sep)

        # Gate GEMMs for BOTH blocks run first: each bank's gate group closes
        # early and its sigmoid drains it while later gate tiles accumulate.
        # The state phases then allocate fresh tiles from the same pool — a
        # 3-generation buffer rotation (gate nb0/nb1, state nb0, state nb1)
        # whose write-after-read waits (sigmoid / previous tanh) all retire
        # well before the new group's first matmul.
        bank, gg, uu, g01 = {}, {}, {}, {}
        for nb in range(NBLK):
            xt, sep = xts[nb], seps[nb]
            for c in range(CT):
                bk = banks.tile([P, NB], f32, tag="bank", name=f"bankG_{nb}_{c}")
                bank[(nb, c)] = bk
                for q in range(QX):
                    lhs = wslice(wg_sb, 2 * q, c)
                    for h in range(2):
                        nc.tensor.matmul(
                            bk[:, h * NH : (h + 1) * NH],
                            lhs,
                            mslice(xt, q, h),
                            start=(q == 0 and h == 0),
                            stop=(q == QX - 1 and h == 1),
                            perf_mode=DR,
                        )
                g = ggpool.tile([P, NB], f16, tag="gg", name=f"gg_{nb}_{c}")
                nc.scalar.activation(g[:], bk[:], AF.Sigmoid, scale=DESCALE)
                gg[(nb, c)] = g
                # Gate-dependent products that do NOT need tanh: u = 0.9*s*g
                # and g01 = 0.1*g; off the critical path, fast 2x fp16 DVE.
                cs = slice(c * NB, (c + 1) * NB)
                u = uupool.tile([P, NB], f16, tag="uu", name=f"uu_{nb}_{c}")
                nc.vector.tensor_tensor(u[:], sep[:, cs], g[:], ALU.mult)
                uu[(nb, c)] = u
                g1 = g1pool.tile([P, NB], f16, tag="gg01", name=f"g01_{nb}_{c}")
                nc.vector.tensor_scalar(g1[:], g[:], 0.1, None, ALU.mult)
                g01[(nb, c)] = g1

        QA = QS - 2 * SG
        for nb in range(NBLK):
            bs = slice(nb * NB, (nb + 1) * NB)
            xt, s_ts = xts[nb], s_tss[nb]
            for c in range(CT):
                bank[(nb, c)] = banks.tile(
                    [P, NB], f32, tag="bank", name=f"bankS_{nb}_{c}"
                )

            # Input GEMM opens the state accumulation groups.  Col-tile-major
            # so col tile c7 (whose bank is freed last by the previous
            # block's tanh) is touched ~3us later than c0.
            for c in range(CT):
                for q in range(QX):
                    lhs = wslice(wi_sb, 2 * q, c)
                    for h in range(2):
                        nc.tensor.matmul(
                            bank[(nb, c)][:, h * NH : (h + 1) * NH],
                            lhs,
                            mslice(xt, q, h),
                            start=(q == 0 and h == 0),
                            stop=False,
                            perf_mode=DR,
                        )

            # Reservoir GEMM part A: q-major over the first 2 chunk groups so
            # consumption tracks the s8/w_res DMA stream.
            for q in range(QA):
                g, qq = divmod(q, SG)
                for c in range(CT):
                    lhs = wslice(wr_sb[g], 2 * qq, c)
                    for h in range(2):
                        nc.tensor.matmul(
                            bank[(nb, c)][:, h * NH : (h + 1) * NH],
                            lhs,
                            mslice(s_ts[g], qq, h),
                            start=False,
                            stop=False,
                            perf_mode=DR,
                        )

            # Part B: last 2 chunk groups col-tile-major, so state banks
            # close staggered (~850ns apart) and each col-tile's epilogue
            # overlaps the remaining matmuls instead of piling up at the end.
            # The very last col tile finishes its two 256-wide halves
            # separately so the first tanh half overlaps the last matmuls.
            for c in range(CT):
                last = nb == NBLK - 1 and c == CT - 1
                hsplit = 2 if last else 1
                for hh in range(hsplit):
                    hr = range(2) if hsplit == 1 else [hh]
                    for q in range(QA, QS):
                        g, qq = divmod(q, SG)
                        lhs = wslice(wr_sb[g], 2 * qq, c)
                        for h in hr:
                            nc.tensor.matmul(
                                bank[(nb, c)][:, h * NH : (h + 1) * NH],
                                lhs,
                                mslice(s_ts[g], qq, h),
                                start=False,
                                stop=(q == QS - 1 and h == 1),
                                perf_mode=DR,
                            )
                for hh in range(hsplit):
                    hs = (
                        slice(0, NB)
                        if hsplit == 1
                        else slice(hh * NH, (hh + 1) * NH)
                    )
                    w = NB if hsplit == 1 else NH
                    tt = ttpool.tile([P, w], f16, tag="tt")
                    nc.scalar.activation(
                        tt[:], bank[(nb, c)][:, hs], AF.Tanh, scale=DESCALE
                    )
                    vv = vvpool.tile([P, w], f16, tag="vv")
                    nc.vector.tensor_tensor(
                        vv[:], tt[:], g01[(nb, c)][:, hs], ALU.mult
                    )
                    nst = nspool.tile(
                        [P, w], f16, tag="ns", name=f"ns_{nb}_{c}_{hh}"
                    )
                    nc.vector.tensor_tensor(
                        nst[:], uu[(nb, c)][:, hs], vv[:], ALU.add
                    )
                    nc.sync.dma_start(
                        ns_out[c * P : (c + 1) * P, bs][:, hs], nst[:]
                    )

    nc.compile()
    return nc


def _get_program():
    if "prog" not in _CACHE:
        _CACHE["prog"] = _build()
    return _CACHE["prog"]


def kernel(inputs, prev_output, reservoir_state, input_weights, reservoir_weights,
           gate_weights):
    import concourse.mybir as mybir
    from concourse.bass_utils import run_bass_kernel_spmd

    nc = _get_program()
    fp8 = mybir.dt.np(mybir.dt.float8e4)

    x = np.ascontiguousarray(np.asarray(inputs, dtype=np.float32))
    s = np.ascontiguousarray(np.asarray(reservoir_state, dtype=np.float32))
    w_in = np.ascontiguousarray(np.asarray(input_weights, dtype=np.float32))
    w_res = np.ascontiguousarray(np.asarray(reservoir_weights, dtype=np.float32))
    w_gate = np.ascontiguousarray(np.asarray(gate_weights, dtype=np.float32))

    xT = x.T  # [D_IN, B]
    sT = s.T  # [R, B]

    def pack_moving(aT):
        # [K, Bh] f32 -> fp8 [K/2, 2*Bh]; row q*128+p,
        # free nb*1024 + jj*512 + i  (see kernel layout comment)
        K, Bh = aT.shape
        a8 = (aT * SCALE_X).astype(fp8)
        a8 = a8.reshape(K // 256, 2, P, Bh // NB, NB)
        return np.ascontiguousarray(
            a8.transpose(0, 2, 3, 1, 4).reshape(K // 2, 2 * Bh)
        )

    def pack_weight(w):
        # [K, COLS] f32 -> fp8 [128, K/128*COLS]; row p, free j*COLS + c
        K = w.shape[0]
        w8 = (w * SCALE_W).astype(fp8)
        return np.ascontiguousarray(
            w8.reshape(K // P, P, COLS).transpose(1, 0, 2).reshape(P, -1)
        )

    # Batch-half moving operands (shared by the 4 column-group cores of
    # each half) and per-column-group weights (shared by both halves).
    x8s = [pack_moving(xT[:, m * B_CORE : (m + 1) * B_CORE]) for m in range(N_BG)]
    s8s = [pack_moving(sT[:, m * B_CORE : (m + 1) * B_CORE]) for m in range(N_BG)]
    wi8s, wr8s, wg8s = [], [], []
    for cg in range(N_CG):
        cslice = slice(cg * COLS, (cg + 1) * COLS)
        wi8s.append(pack_weight(w_in[:, cslice]))
        wr8s.append(pack_weight(w_res[:, cslice]))
        wg8s.append(pack_weight(w_gate[:, cslice]))

    in_maps = []
    for core in range(N_CORES):
        m, cg = divmod(core, N_CG)
        cslice = slice(cg * COLS, (cg + 1) * COLS)
        bslice = slice(m * B_CORE, (m + 1) * B_CORE)
        in_maps.append(
            {
                "x8": x8s[m],
                "s8": s8s[m],
                "w_in8": wi8s[cg],
                "w_res8": wr8s[cg],
                "w_gate8": wg8s[cg],
                "sepi": np.ascontiguousarray(
                    (0.9 * sT[cslice, bslice]).astype(np.float16)
                ),
            }
        )

    res = run_bass_kernel_spmd(nc, in_maps, list(range(N_CORES)))

    ns_T = np.empty((R, B), dtype=np.float16)
    for core in range(N_CORES):
        m, cg = divmod(core, N_CG)
        ns_T[cg * COLS : (cg + 1) * COLS, m * B_CORE : (m + 1) * B_CORE] = (
            np.asarray(res.results[core]["ns_out"])
        )
    new_state = np.ascontiguousarray(ns_T.T.astype(np.float32))  # [B, R]
    output = (new_state > 0.5).astype(np.float32)

    # fp8 matmuls carry ~4e-3 RMS noise into new_state; elements within
    # BAND of the spike threshold are re-evaluated exactly (f64 accumulation
    # over the true f32 inputs) and both outputs patched.
    bi, rj = np.nonzero(np.abs(new_state - 0.5) < BAND)
    for lo in range(0, bi.size, 32768):
        bc = bi[lo : lo + 32768]
        rc = rj[lo : lo + 32768]
        xg = x[bc]
        acc = np.einsum("ij,ji->i", xg, w_in[:, rc], dtype=np.float64,
                        optimize=True)
        acc += np.einsum("ij,ji->i", s[bc], w_res[:, rc], dtype=np.float64,
                         optimize=True)
        g_acc = np.einsum("ij,ji->i", xg, w_gate[:, rc], dtype=np.float64,
                          optimize=True)
        gate_v = 1.0 / (1.0 + np.exp(-g_acc))
        ns_fix = (0.9 * s[bc, rc].astype(np.float64)
                  + 0.1 * np.tanh(acc)) * gate_v
        ns_fix32 = ns_fix.astype(np.float32)
        new_state[bc, rc] = ns_fix32
        output[bc, rc] = (ns_fix32 > 0.5).astype(np.float32)
    return output, new_state


# revision 31
# speedup vs baseline: 3.5804x; 1.0542x over previous
"""Gated spiking reservoir step — Trainium2 Bass kernel (8 NeuronCores).

Math (per reference):
    ic   = inputs @ input_weights                  # [B, R]
    rc   = reservoir_state @ reservoir_weights     # [B, R]
    gate = sigmoid(inputs @ gate_weights)          # [B, R]
    ns   = (0.9 * reservoir_state + 0.1 * tanh(ic + rc)) * gate
    out  = (ns > 0.5) ? 1.0 : 0.0
    returns (out, ns)

Sharding: hybrid tensor/data parallel — 4 column groups x 2 batch halves.
Core (m, cg) owns a 1024-column slice of the three weight matrices and a
1024-row batch half, producing the [1024-col x 1024-batch] tile of both
outputs.  This minimizes per-core HBM traffic (~15 MB: weights 6 +
activations 5 + epilogue-state 2 + output 2).

All three GEMMs run in fp8 (e4m3) with the PE's DoubleRow perf mode: one
matmul instruction contracts 256 rows (2 fp8 rows per partition) at 0.5
cycles/row — 4x the fp32r rate.  Operands are pre-scaled on host (x*16,
s*16, w*512) so fp8 quantization lands in e4m3's normal range; the 1/8192
descale folds into the tanh/sigmoid activation scale.

The batch half is processed in 2 blocks of 512.  PSUM: 8 banks, one per
column tile of 128; within a block each bank first accumulates the gate
GEMM (group closed early, sigmoid drains it), then the state GEMM
(input + reservoir) under a second accumulation group.

Epilogue per column tile: sigmoid/tanh on ScalarE (from PSUM, descaled),
then fast 2x-mode fp16 VectorE ops: u = (0.9s)*g and g01 = 0.1*g are
issued right after the sigmoid (off the critical path; host pre-scales
sepi by 0.9), leaving only m = tanh*g01 and ns = u + m after the last
matmul.  fp16 transposed output DMAs per column tile.

Host side: assemble/transpose, spike threshold, and an exact float64
re-evaluation of elements within BAND of the 0.5 threshold (fp8 matmul
noise ~4e-3 RMS would otherwise flip spikes near the boundary).
"""

import sys

if "/opt/trn_rl_repo" not in sys.path:
    sys.path.insert(0, "/opt/trn_rl_repo")

import numpy as np

B, D_IN, R = 2048, 1024, 4096
N_CORES = 8
N_CG = 4                     # column groups
N_BG = 2                     # batch groups
COLS = R // N_CG             # 1024 output columns per core
B_CORE = B // N_BG           # 1024 batch rows per core
P = 128                      # SBUF/PSUM partitions
CT = COLS // P               # 8 column tiles per core
NB = 512                     # batch block (one PSUM bank of f32)
NH = NB // 2                 # moving free dim per DoubleRow matmul
NBLK = B_CORE // NB          # 2 batch blocks
QX = D_IN // 256             # 4 double-row contraction tiles over input dim
QS = R // 256                # 16 double-row contraction tiles over reservoir
SG = 4                       # s8/w_res chunk granularity: 4 q-tiles

SCALE_X = 16.0               # fp8 pre-scale for activations (x and s)
SCALE_W = 512.0              # fp8 pre-scale for weights
DESCALE = 1.0 / (SCALE_X * SCALE_W)

# Re-evaluate elements this close to the 0.5 spike threshold exactly on
# host (fp8 matmul noise sigma ~4e-3; 0.018 ~ 4.5 sigma).
BAND = 0.018

_CACHE = {}


def _build():
    from contextlib import ExitStack

    from concourse import bacc, tile
    import concourse.mybir as mybir

    f32 = mybir.dt.float32
    f16 = mybir.dt.float16
    fp8 = mybir.dt.float8e4
    AF = mybir.ActivationFunctionType
    ALU = mybir.AluOpType
    DR = mybir.MatmulPerfMode.DoubleRow

    nc = bacc.Bacc(
        "TRN2", target_bir_lowering=False, debug=False, enable_asserts=False
    )

    # DRAM layouts (host pre-packed so every DMA is contiguous >=512B/row):
    #   x8   [512, 2048]  row q*128+p, free nb*1024 + jj*512 + i
    #                     = fp8(16 * x.T)[q*256+jj*128+p, bg*1024 + nb*512+i]
    #   s8   [2048, 2048] same layout over the 4096 reservoir rows
    #   w_*8 [128, J*COLS] row p, free j*COLS + c = fp8(512*w)[j*128+p, c]
    #   sepi [1024, 1024] fp16 (0.9 * s.T)[cols, batch-half]
    #   ns   [1024, 1024] fp16 output, same [col, batch] layout
    x8 = nc.dram_tensor("x8", [4 * P, NBLK * 1024], fp8, kind="ExternalInput")
    s8 = nc.dram_tensor("s8", [16 * P, NBLK * 1024], fp8, kind="ExternalInput")
    w_in8 = nc.dram_tensor("w_in8", [P, 8 * COLS], fp8, kind="ExternalInput")
    w_res8 = nc.dram_tensor("w_res8", [P, 32 * COLS], fp8, kind="ExternalInput")
    w_gate8 = nc.dram_tensor("w_gate8", [P, 8 * COLS], fp8, kind="ExternalInput")
    sepi = nc.dram_tensor("sepi", [COLS, B_CORE], f16, kind="ExternalInput")
    ns_out = nc.dram_tensor("ns_out", [COLS, B_CORE], f16, kind="ExternalOutput")

    def wslice(t, j2, ct):
        # [128, 2, 128] stationary view: rows {j2, j2+1}, cols ct*128..+128
        return t[:, j2 * COLS : (j2 + 2) * COLS].rearrange(
            "p (j c) -> p j c", j=2
        )[:, :, ct * P : (ct + 1) * P]

    def mslice(t, q, h):
        # [128, 2, 256] moving view from a flat [128, q*1024] block tile
        return t[:, q * 1024 : (q + 1) * 1024].rearrange(
            "p (j i) -> p j i", j=2
        )[:, :, h * NH : (h + 1) * NH]

    with tile.TileContext(nc) as tc, ExitStack() as ctx:
        wpool = ctx.enter_context(tc.tile_pool(name="weights", bufs=1))
        wg_sb = wpool.tile([P, 8 * COLS], fp8, tag="wg", name="wg_sb")
        wi_sb = wpool.tile([P, 8 * COLS], fp8, tag="wi", name="wi_sb")
        wr_sb = [
            wpool.tile([P, 2 * SG * COLS], fp8, tag=f"wr{g}", name=f"wr_sb{g}")
            for g in range(QS // SG)
        ]

        xpool = ctx.enter_context(tc.tile_pool(name="x_mov", bufs=NBLK))
        spool = ctx.enter_context(tc.tile_pool(name="s_mov", bufs=QS // SG * NBLK))
        sepool = ctx.enter_context(tc.tile_pool(name="s_epi", bufs=NBLK))
        banks = ctx.enter_context(tc.tile_pool(name="banks", bufs=CT, space="PSUM"))
        ggpool = ctx.enter_context(tc.tile_pool(name="gg", bufs=2 * CT))
        g1pool = ctx.enter_context(tc.tile_pool(name="gg01", bufs=2 * CT))
        uupool = ctx.enter_context(tc.tile_pool(name="uu", bufs=2 * CT))
        ttpool = ctx.enter_context(tc.tile_pool(name="tt", bufs=CT))
        vvpool = ctx.enter_context(tc.tile_pool(name="vv", bufs=CT))
        nspool = ctx.enter_context(tc.tile_pool(name="ns", bufs=2 * CT))

        # Everything is SBUF-resident (~14 MB): issue ALL input DMAs upfront
        # in consumption order (the first matmuls need half of w_gate plus
        # block 0's x, so those two lead), and the SP queue never head-of-line
        # blocks a load behind an output store that waits on compute.
        xts, seps, s_tss = [], [], []

        def load_x(nb, half=None):
            if half is None:
                xt = xpool.tile([P, QX * 1024], fp8, tag="x", name=f"x_{nb}")
                xts.append(xt)
                qr = slice(0, QX)
            else:
                if half == 0:
                    xt = xpool.tile([P, QX * 1024], fp8, tag="x",
                                    name=f"x_{nb}")
                    xts.append(xt)
                xt = xts[nb]
                qr = slice(half * QX // 2, (half + 1) * QX // 2)
            nc.sync.dma_start(
                xt[:].rearrange("p (q f) -> p q f", q=QX)[:, qr],
                x8[:, nb * 1024 : (nb + 1) * 1024].rearrange(
                    "(q p) f -> p q f", p=P
                )[:, qr],
            )

        def load_wg_half(k):
            hw = COLS // 2
            nc.sync.dma_start(
                wg_sb[:].rearrange("p (j c) -> p j c", j=8)
                [:, :, k * hw : (k + 1) * hw],
                w_gate8[:, :].rearrange("p (j c) -> p j c", j=8)
                [:, :, k * hw : (k + 1) * hw],
            )

        # Bootstrap: col tile 0's gate weights into a small separate tile
        # (~128 KB total) so the very first matmul issues ~2us earlier than
        # waiting for the 0.5 MB w_gate half (which follows unchanged; col
        # tile 0 reads the bootstrap tile instead).
        wg_c0 = wpool.tile([P, 8 * P], fp8, tag="wg_c0", name="wg_c0")
        for q in range(QX):
            nc.sync.dma_start(
                wg_c0[:].rearrange("p (j c) -> p j c", j=8)[
                    :, 2 * q : 2 * q + 2, :
                ],
                w_gate8[:, :].rearrange("p (j c) -> p j c", j=8)[
                    :, 2 * q : 2 * q + 2, :P
                ],
            )
        load_x(0, 0)
        load_x(0, 1)
        load_wg_half(0)
        load_wg_half(1)
        load_x(1)
        nc.sync.dma_start(wi_sb[:], w_in8[:, :])
        for nb in range(NBLK):
            bs1k = slice(nb * 1024, (nb + 1) * 1024)
            bs = slice(nb * NB, (nb + 1) * NB)
            s_ts = []
            for g in range(QS // SG):
                if nb == 0:
                    nc.sync.dma_start(
                        wr_sb[g][:],
                        w_res8[:, g * 2 * SG * COLS : (g + 1) * 2 * SG * COLS],
                    )
                st = spool.tile([P, SG * 1024], fp8, tag="s8", name=f"s8_{nb}_{g}")
                nc.sync.dma_start(
                    st[:].rearrange("p (q f) -> p q f", q=SG),
                    s8[g * SG * P : (g + 1) * SG * P, bs1k].rearrange(
                        "(q p) f -> p q f", p=P
                    ),
                )
                s_ts.append(st)
            s_tss.append(s_ts)
            sep = sepool.tile([P, CT * NB], f16, tag="sepi", name=f"sepi_{nb}")
            nc.sync.dma_start(
                sep[:].rearrange("p (c f) -> p c f", c=CT),
                sepi[:, bs].rearrange("(c p) f -> p c f", p=P),
            )
            seps.append(sep)

        # Gate GEMMs for BOTH blocks run first: each bank's gate group closes
        # early and its sigmoid drains it while later gate tiles accumulate.
        # The state phases then allocate fresh tiles from the same pool — a
        # 3-generation buffer rotation (gate nb0/nb1, state nb0, state nb1)
        # whose write-after-read waits (sigmoid / previous tanh) all retire
        # well before the new group's first matmul.
        bank, gg, uu, g01 = {}, {}, {}, {}
        for nb in range(NBLK):
            xt, sep = xts[nb], seps[nb]
            for c in range(CT):
                bk = banks.tile([P, NB], f32, tag="bank", name=f"bankG_{nb}_{c}")
                bank[(nb, c)] = bk
                for q in range(QX):
                    lhs = wslice(wg_sb, 2 * q, c)
                    for h in range(2):
                        nc.tensor.matmul(
                            bk[:, h * NH : (h + 1) * NH],
                            lhs,
                            mslice(xt, q, h),
                            start=(q == 0 and h == 0),
                            stop=(q == QX - 1 and h == 1),
                            perf_mode=DR,
                        )
                g = ggpool.tile([P, NB], f16, tag="gg", name=f"gg_{nb}_{c}")
                nc.scalar.activation(g[:], bk[:], AF.Sigmoid, scale=DESCALE)
                gg[(nb, c)] = g
                # Gate-dependent products that do NOT need tanh: u = 0.9*s*g
                # and g01 = 0.1*g; off the critical path, fast 2x fp16 DVE.
                cs = slice(c * NB, (c + 1) * NB)
                u = uupool.tile([P, NB], f16, tag="uu", name=f"uu_{nb}_{c}")
                nc.vector.tensor_tensor(u[:], sep[:, cs], g[:], ALU.mult)
                uu[(nb, c)] = u
                g1 = g1pool.tile([P, NB], f16, tag="gg01", name=f"g01_{nb}_{c}")
                nc.vector.tensor_scalar(g1[:], g[:], 0.1, None, ALU.mult)
                g01[(nb, c)] = g1

        QA = QS - 2 * SG
        for nb in range(NBLK):
            bs = slice(nb * NB, (nb + 1) * NB)
            xt, s_ts = xts[nb], s_tss[nb]
            for c in range(CT):
                bank[(nb, c)] = banks.tile(
                    [P, NB], f32, tag="bank", name=f"bankS_{nb}_{c}"
                )

            # Input GEMM opens the state accumulation groups.  Col-tile-major
            # so col tile c7 (whose bank is freed last by the previous
            # block's tanh) is touched ~3us later than c0.
            for c in range(CT):
                for q in range(QX):
                    lhs = wslice(wi_sb, 2 * q, c)
                    for h in range(2):
                        nc.tensor.matmul(
                            bank[(nb, c)][:, h * NH : (h + 1) * NH],
                            lhs,
                            mslice(xt, q, h),
                            start=(q == 0 and h == 0),
                            stop=False,
                            perf_mode=DR,
                        )

            # Reservoir GEMM part A: q-major over the first 2 chunk groups so
            # consumption tracks the s8/w_res DMA stream.
            for q in range(QA):
                g, qq = divmod(q, SG)
                for c in range(CT):
                    lhs = wslice(wr_sb[g], 2 * qq, c)
                    for h in range(2):
                        nc.tensor.matmul(
                            bank[(nb, c)][:, h * NH : (h + 1) * NH],
                            lhs,
                            mslice(s_ts[g], qq, h),
                            start=False,
                            stop=False,
                            perf_mode=DR,
                        )

            # Part B: last 2 chunk groups col-tile-major, so state banks
            # close staggered (~850ns apart) and each col-tile's epilogue
            # overlaps the remaining matmuls instead of piling up at the end.
            # Outputs are merged per col-tile PAIR (one DMA per 2 tiles) so
            # the tail doesn't serialize on per-DMA HWDGE overhead.
            pair = None
            for c in range(CT):
                last = nb == NBLK - 1 and c == CT - 1
                hsplit = 2 if last else 1
                for hh in range(hsplit):
                    hr = range(2) if hsplit == 1 else [hh]
                    for q in range(QA, QS):
                        g, qq = divmod(q, SG)
                        lhs = wslice(wr_sb[g], 2 * qq, c)
                        for h in hr:
                            nc.tensor.matmul(
                                bank[(nb, c)][:, h * NH : (h + 1) * NH],
                                lhs,
                                mslice(s_ts[g], qq, h),
                                start=False,
                                stop=(q == QS - 1 and h == 1),
                                perf_mode=DR,
                            )
                tail = nb == NBLK - 1 and c >= CT - 2
                if pair is None:
                    pair = nspool.tile(
                        [P, 2 * NB], f16, tag="ns", name=f"ns_{nb}_{c}"
                    )
                po = (c % 2) * NB
                for hh in range(hsplit):
                    nq = 1
                    for qq2 in range(nq):
                        w = NB // (hsplit * nq)
                        lo = (hh * nq + qq2) * w
                        hs = slice(lo, lo + w)
                        tt = ttpool.tile([P, w], f16, tag="tt")
                        nc.scalar.activation(
                            tt[:], bank[(nb, c)][:, hs], AF.Tanh, scale=DESCALE
                        )
                        vv = vvpool.tile([P, w], f16, tag="vv")
                        nc.vector.tensor_tensor(
                            vv[:], tt[:], g01[(nb, c)][:, hs], ALU.mult
                        )
                        nc.vector.tensor_tensor(
                            pair[:, po + hs.start : po + hs.stop],
                            uu[(nb, c)][:, hs],
                            vv[:],
                            ALU.add,
                        )
                        if tail:
                            # Final two col tiles ship piecewise; the very
                            # last transfer is only 32 KB, minimizing the
                            # post-kernel DMA+semaphore tail.
                            nc.sync.dma_start(
                                ns_out[c * P : (c + 1) * P, bs][:, hs],
                                pair[:, po + hs.start : po + hs.stop],
                            )
                if c % 2 == 1:
                    if not tail:
                        nc.sync.dma_start(
                            ns_out[(c - 1) * P : (c + 1) * P, bs].rearrange(
                                "(c p) f -> p c f", p=P
                            ),
                            pair[:].rearrange("p (c f) -> p c f", c=2),
                        )
                    pair = None

    nc.compile()
    return nc


def _get_program():
    if "prog" not in _CACHE:
        _CACHE["prog"] = _build()
    return _CACHE["prog"]


def kernel(inputs, prev_output, reservoir_state, input_weights, reservoir_weights,
           gate_weights):
    import concourse.mybir as mybir
    from concourse.bass_utils import run_bass_kernel_spmd

    nc = _get_program()
    fp8 = mybir.dt.np(mybir.dt.float8e4)

    x = np.ascontiguousarray(np.asarray(inputs, dtype=np.float32))
    s = np.ascontiguousarray(np.asarray(reservoir_state, dtype=np.float32))
    w_in = np.ascontiguousarray(np.asarray(input_weights, dtype=np.float32))
    w_res = np.ascontiguousarray(np.asarray(reservoir_weights, dtype=np.float32))
    w_gate = np.ascontiguousarray(np.asarray(gate_weights, dtype=np.float32))

    xT = x.T  # [D_IN, B]
    sT = s.T  # [R, B]

    def pack_moving(aT):
        # [K, Bh] f32 -> fp8 [K/2, 2*Bh]; row q*128+p,
        # free nb*1024 + jj*512 + i  (see kernel layout comment)
        K, Bh = aT.shape
        a8 = (aT * SCALE_X).astype(fp8)
        a8 = a8.reshape(K // 256, 2, P, Bh // NB, NB)
        return np.ascontiguousarray(
            a8.transpose(0, 2, 3, 1, 4).reshape(K // 2, 2 * Bh)
        )

    def pack_weight(w):
        # [K, COLS] f32 -> fp8 [128, K/128*COLS]; row p, free j*COLS + c
        K = w.shape[0]
        w8 = (w * SCALE_W).astype(fp8)
        return np.ascontiguousarray(
            w8.reshape(K // P, P, COLS).transpose(1, 0, 2).reshape(P, -1)
        )

    # Batch-half moving operands (shared by the 4 column-group cores of
    # each half) and per-column-group weights (shared by both halves).
    x8s = [pack_moving(xT[:, m * B_CORE : (m + 1) * B_CORE]) for m in range(N_BG)]
    s8s = [pack_moving(sT[:, m * B_CORE : (m + 1) * B_CORE]) for m in range(N_BG)]
    wi8s, wr8s, wg8s = [], [], []
    for cg in range(N_CG):
        cslice = slice(cg * COLS, (cg + 1) * COLS)
        wi8s.append(pack_weight(w_in[:, cslice]))
        wr8s.append(pack_weight(w_res[:, cslice]))
        wg8s.append(pack_weight(w_gate[:, cslice]))

    in_maps = []
    for core in range(N_CORES):
        m, cg = divmod(core, N_CG)
        cslice = slice(cg * COLS, (cg + 1) * COLS)
        bslice = slice(m * B_CORE, (m + 1) * B_CORE)
        in_maps.append(
            {
                "x8": x8s[m],
                "s8": s8s[m],
                "w_in8": wi8s[cg],
                "w_res8": wr8s[cg],
                "w_gate8": wg8s[cg],
                "sepi": np.ascontiguousarray(
                    (0.9 * sT[cslice, bslice]).astype(np.float16)
                ),
            }
        )

    res = run_bass_kernel_spmd(nc, in_maps, list(range(N_CORES)))

    ns_T = np.empty((R, B), dtype=np.float16)
    for core in range(N_CORES):
        m, cg = divmod(core, N_CG)
        ns_T[cg * COLS : (cg + 1) * COLS, m * B_CORE : (m + 1) * B_CORE] = (
            np.asarray(res.results[core]["ns_out"])
        )
    new_state = np.ascontiguousarray(ns_T.T.astype(np.float32))  # [B, R]
    output = (new_state > 0.5).astype(np.float32)

    # fp8 matmuls carry ~4e-3 RMS noise into new_state; elements within
    # BAND of the spike threshold are re-evaluated exactly (f64 accumulation
    # over the true f32 inputs) and both outputs patched.
    bi, rj = np.nonzero(np.abs(new_state - 0.5) < BAND)
    for lo in range(0, bi.size, 32768):
        bc = bi[lo : lo + 32768]
        rc = rj[lo : lo + 32768]
        xg = x[bc]
        acc = np.einsum("ij,ji->i", xg, w_in[:, rc], dtype=np.float64,
                        optimize=True)
        acc += np.einsum("ij,ji->i", s[bc], w_res[:, rc], dtype=np.float64,
                         optimize=True)
        g_acc = np.einsum("ij,ji->i", xg, w_gate[:, rc], dtype=np.float64,
                          optimize=True)
        gate_v = 1.0 / (1.0 + np.exp(-g_acc))
        ns_fix = (0.9 * s[bc, rc].astype(np.float64)
                  + 0.1 * np.tanh(acc)) * gate_v
        ns_fix32 = ns_fix.astype(np.float32)
        new_state[bc, rc] = ns_fix32
        output[bc, rc] = (ns_fix32 > 0.5).astype(np.float32)
    return output, new_state


# revision 38
# speedup vs baseline: 3.5955x; 1.0042x over previous
"""Gated spiking reservoir step — Trainium2 Bass kernel (8 NeuronCores).

Math (per reference):
    ic   = inputs @ input_weights                  # [B, R]
    rc   = reservoir_state @ reservoir_weights     # [B, R]
    gate = sigmoid(inputs @ gate_weights)          # [B, R]
    ns   = (0.9 * reservoir_state + 0.1 * tanh(ic + rc)) * gate
    out  = (ns > 0.5) ? 1.0 : 0.0
    returns (out, ns)

Sharding: hybrid tensor/data parallel — 4 column groups x 2 batch halves.
Core (m, cg) owns a 1024-column slice of the three weight matrices and a
1024-row batch half, producing the [1024-col x 1024-batch] tile of both
outputs.  This minimizes per-core HBM traffic (~15 MB: weights 6 +
activations 5 + epilogue-state 2 + output 2).

All three GEMMs run in fp8 (e4m3) with the PE's DoubleRow perf mode: one
matmul instruction contracts 256 rows (2 fp8 rows per partition) at 0.5
cycles/row — 4x the fp32r rate.  Operands are pre-scaled on host (x*16,
s*16, w*512) so fp8 quantization lands in e4m3's normal range; the 1/8192
descale folds into the tanh/sigmoid activation scale.

The batch half is processed in 2 blocks of 512.  PSUM: 8 banks, one per
column tile of 128, in a 3-generation rotation: the gate GEMMs of BOTH
blocks run first (each group closed early, its sigmoid drains the bank),
then each block's state GEMM (input + reservoir) opens a fresh
accumulation group on the same bank.  All input DMAs are issued upfront
in consumption order (everything is SBUF-resident, ~14 MB), so the
schedule is paced purely by HBM bandwidth and PE throughput.

Epilogue per column tile: sigmoid/tanh on ScalarE (from PSUM, descaled),
then fast 2x-mode fp16 VectorE ops: u = (0.9s)*g and g01 = 0.1*g are
issued right after the sigmoid (off the critical path; host pre-scales
sepi by 0.9), leaving only m = tanh*g01 and ns = u + m after the last
matmul.  The reservoir GEMM's last 2 chunk groups run col-tile-major so
the per-tile epilogues overlap the remaining matmuls; fp16 outputs ship
as one transposed DMA per col-tile pair, piecewise for the final tile.

Host side: assemble/transpose, spike threshold, and an exact float64
re-evaluation of elements within BAND of the 0.5 threshold (fp8 matmul
noise ~4e-3 RMS would otherwise flip spikes near the boundary).
"""

import sys

if "/opt/trn_rl_repo" not in sys.path:
    sys.path.insert(0, "/opt/trn_rl_repo")

import numpy as np

B, D_IN, R = 2048, 1024, 4096
N_CORES = 8
N_CG = 4                     # column groups
N_BG = 2                     # batch groups
COLS = R // N_CG             # 1024 output columns per core
B_CORE = B // N_BG           # 1024 batch rows per core
P = 128                      # SBUF/PSUM partitions
CT = COLS // P               # 8 column tiles per core
NB = 512                     # batch block (one PSUM bank of f32)
NH = NB // 2                 # moving free dim per DoubleRow matmul
NBLK = B_CORE // NB          # 2 batch blocks
QX = D_IN // 256             # 4 double-row contraction tiles over input dim
QS = R // 256                # 16 double-row contraction tiles over reservoir
SG = 4                       # s8/w_res chunk granularity: 4 q-tiles

SCALE_X = 16.0               # fp8 pre-scale for activations (x and s)
SCALE_W = 512.0              # fp8 pre-scale for weights
DESCALE = 1.0 / (SCALE_X * SCALE_W)

# Re-evaluate elements this close to the 0.5 spike threshold exactly on
# host (fp8 matmul noise sigma ~4e-3; 0.018 ~ 4.5 sigma).
BAND = 0.018

_CACHE = {}


def _build():
    from contextlib import ExitStack

    from concourse import bacc, tile
    import concourse.mybir as mybir

    f32 = mybir.dt.float32
    f16 = mybir.dt.float16
    fp8 = mybir.dt.float8e4
    AF = mybir.ActivationFunctionType
    ALU = mybir.AluOpType
    DR = mybir.MatmulPerfMode.DoubleRow

    nc = bacc.Bacc(
        "TRN2", target_bir_lowering=False, debug=False, enable_asserts=False
    )

    # DRAM layouts (host pre-packed so every DMA is contiguous >=512B/row):
    #   x8   [512, 2048]  row q*128+p, free nb*1024 + jj*512 + i
    #                     = fp8(16 * x.T)[q*256+jj*128+p, bg*1024 + nb*512+i]
    #   s8   [2048, 2048] same layout over the 4096 reservoir rows
    #   w_*8 [128, J*COLS] row p, free j*COLS + c = fp8(512*w)[j*128+p, c]
    #   sepi [1024, 1024] fp16 (0.9 * s.T)[cols, batch-half]
    #   ns   [1024, 1024] fp16 output, same [col, batch] layout
    x8 = nc.dram_tensor("x8", [4 * P, NBLK * 1024], fp8, kind="ExternalInput")
    s8 = nc.dram_tensor("s8", [16 * P, NBLK * 1024], fp8, kind="ExternalInput")
    w_in8 = nc.dram_tensor("w_in8", [P, 8 * COLS], fp8, kind="ExternalInput")
    w_res8 = nc.dram_tensor("w_res8", [P, 32 * COLS], fp8, kind="ExternalInput")
    w_gate8 = nc.dram_tensor("w_gate8", [P, 8 * COLS], fp8, kind="ExternalInput")
    sepi = nc.dram_tensor("sepi", [COLS, B_CORE], f16, kind="ExternalInput")
    ns_out = nc.dram_tensor("ns_out", [COLS, B_CORE], f16, kind="ExternalOutput")

    def wslice(t, j2, ct):
        # [128, 2, 128] stationary view: rows {j2, j2+1}, cols ct*128..+128
        return t[:, j2 * COLS : (j2 + 2) * COLS].rearrange(
            "p (j c) -> p j c", j=2
        )[:, :, ct * P : (ct + 1) * P]

    def mslice(t, q, h):
        # [128, 2, 256] moving view from a flat [128, q*1024] block tile
        return t[:, q * 1024 : (q + 1) * 1024].rearrange(
            "p (j i) -> p j i", j=2
        )[:, :, h * NH : (h + 1) * NH]

    with tile.TileContext(nc) as tc, ExitStack() as ctx:
        wpool = ctx.enter_context(tc.tile_pool(name="weights", bufs=1))
        wg_sb = wpool.tile([P, 8 * COLS], fp8, tag="wg", name="wg_sb")
        wi_sb = wpool.tile([P, 8 * COLS], fp8, tag="wi", name="wi_sb")
        wr_sb = [
            wpool.tile([P, 2 * SG * COLS], fp8, tag=f"wr{g}", name=f"wr_sb{g}")
            for g in range(QS // SG)
        ]

        xpool = ctx.enter_context(tc.tile_pool(name="x_mov", bufs=NBLK))
        spool = ctx.enter_context(tc.tile_pool(name="s_mov", bufs=QS // SG * NBLK))
        sepool = ctx.enter_context(tc.tile_pool(name="s_epi", bufs=NBLK))
        banks = ctx.enter_context(tc.tile_pool(name="banks", bufs=CT, space="PSUM"))
        ggpool = ctx.enter_context(tc.tile_pool(name="gg", bufs=2 * CT))
        g1pool = ctx.enter_context(tc.tile_pool(name="gg01", bufs=2 * CT))
        uupool = ctx.enter_context(tc.tile_pool(name="uu", bufs=2 * CT))
        ttpool = ctx.enter_context(tc.tile_pool(name="tt", bufs=CT))
        vvpool = ctx.enter_context(tc.tile_pool(name="vv", bufs=CT))
        nspool = ctx.enter_context(tc.tile_pool(name="ns", bufs=2 * CT))

        # Everything is SBUF-resident (~14 MB): issue ALL input DMAs upfront
        # in consumption order (the first matmuls need half of w_gate plus
        # block 0's x, so those two lead), and the SP queue never head-of-line
        # blocks a load behind an output store that waits on compute.
        xts, seps, s_tss = [], [], []

        def load_x(nb, half=None):
            if half is None:
                xt = xpool.tile([P, QX * 1024], fp8, tag="x", name=f"x_{nb}")
                xts.append(xt)
                qr = slice(0, QX)
            else:
                if half == 0:
                    xt = xpool.tile([P, QX * 1024], fp8, tag="x",
                                    name=f"x_{nb}")
                    xts.append(xt)
                xt = xts[nb]
                qr = slice(half * QX // 2, (half + 1) * QX // 2)
            nc.sync.dma_start(
                xt[:].rearrange("p (q f) -> p q f", q=QX)[:, qr],
                x8[:, nb * 1024 : (nb + 1) * 1024].rearrange(
                    "(q p) f -> p q f", p=P
                )[:, qr],
            )

        def load_wg_half(k):
            hw = COLS // 2
            nc.sync.dma_start(
                wg_sb[:].rearrange("p (j c) -> p j c", j=8)
                [:, :, k * hw : (k + 1) * hw],
                w_gate8[:, :].rearrange("p (j c) -> p j c", j=8)
                [:, :, k * hw : (k + 1) * hw],
            )

        load_wg_half(0)
        load_x(0, 0)
        load_x(0, 1)
        load_wg_half(1)
        load_x(1)
        nc.sync.dma_start(wi_sb[:], w_in8[:, :])
        for nb in range(NBLK):
            bs1k = slice(nb * 1024, (nb + 1) * 1024)
            bs = slice(nb * NB, (nb + 1) * NB)
            s_ts = []
            for g in range(QS // SG):
                if nb == 0:
                    nc.sync.dma_start(
                        wr_sb[g][:],
                        w_res8[:, g * 2 * SG * COLS : (g + 1) * 2 * SG * COLS],
                    )
                st = spool.tile([P, SG * 1024], fp8, tag="s8", name=f"s8_{nb}_{g}")
                nc.sync.dma_start(
                    st[:].rearrange("p (q f) -> p q f", q=SG),
                    s8[g * SG * P : (g + 1) * SG * P, bs1k].rearrange(
                        "(q p) f -> p q f", p=P
                    ),
                )
                s_ts.append(st)
            s_tss.append(s_ts)
            sep = sepool.tile([P, CT * NB], f16, tag="sepi", name=f"sepi_{nb}")
            nc.sync.dma_start(
                sep[:].rearrange("p (c f) -> p c f", c=CT),
                sepi[:, bs].rearrange("(c p) f -> p c f", p=P),
            )
            seps.append(sep)

        # Gate GEMMs for BOTH blocks run first: each bank's gate group closes
        # early and its sigmoid drains it while later gate tiles accumulate.
        # The state phases then allocate fresh tiles from the same pool — a
        # 3-generation buffer rotation (gate nb0/nb1, state nb0, state nb1)
        # whose write-after-read waits (sigmoid / previous tanh) all retire
        # well before the new group's first matmul.
        bank, gg, uu, g01 = {}, {}, {}, {}
        for nb in range(NBLK):
            xt, sep = xts[nb], seps[nb]
            for c in range(CT):
                bk = banks.tile([P, NB], f32, tag="bank", name=f"bankG_{nb}_{c}")
                bank[(nb, c)] = bk
                for q in range(QX):
                    lhs = wslice(wg_sb, 2 * q, c)
                    for h in range(2):
                        nc.tensor.matmul(
                            bk[:, h * NH : (h + 1) * NH],
                            lhs,
                            mslice(xt, q, h),
                            start=(q == 0 and h == 0),
                            stop=(q == QX - 1 and h == 1),
                            perf_mode=DR,
                        )
                g = ggpool.tile([P, NB], f16, tag="gg", name=f"gg_{nb}_{c}")
                nc.scalar.activation(g[:], bk[:], AF.Sigmoid, scale=DESCALE)
                gg[(nb, c)] = g
                # Gate-dependent products that do NOT need tanh: u = 0.9*s*g
                # and g01 = 0.1*g; off the critical path, fast 2x fp16 DVE.
                cs = slice(c * NB, (c + 1) * NB)
                u = uupool.tile([P, NB], f16, tag="uu", name=f"uu_{nb}_{c}")
                nc.vector.tensor_tensor(u[:], sep[:, cs], g[:], ALU.mult)
                uu[(nb, c)] = u
                g1 = g1pool.tile([P, NB], f16, tag="gg01", name=f"g01_{nb}_{c}")
                nc.vector.tensor_scalar(g1[:], g[:], 0.1, None, ALU.mult)
                g01[(nb, c)] = g1

        QA = QS - 2 * SG
        for nb in range(NBLK):
            bs = slice(nb * NB, (nb + 1) * NB)
            xt, s_ts = xts[nb], s_tss[nb]
            for c in range(CT):
                bank[(nb, c)] = banks.tile(
                    [P, NB], f32, tag="bank", name=f"bankS_{nb}_{c}"
                )

            # Input GEMM opens the state accumulation groups.  Col-tile-major
            # so col tile c7 (whose bank is freed last by the previous
            # block's tanh) is touched ~3us later than c0.
            for c in range(CT):
                for q in range(QX):
                    lhs = wslice(wi_sb, 2 * q, c)
                    for h in range(2):
                        nc.tensor.matmul(
                            bank[(nb, c)][:, h * NH : (h + 1) * NH],
                            lhs,
                            mslice(xt, q, h),
                            start=(q == 0 and h == 0),
                            stop=False,
                            perf_mode=DR,
                        )

            # Reservoir GEMM part A: q-major over the first 2 chunk groups so
            # consumption tracks the s8/w_res DMA stream.
            for q in range(QA):
                g, qq = divmod(q, SG)
                for c in range(CT):
                    lhs = wslice(wr_sb[g], 2 * qq, c)
                    for h in range(2):
                        nc.tensor.matmul(
                            bank[(nb, c)][:, h * NH : (h + 1) * NH],
                            lhs,
                            mslice(s_ts[g], qq, h),
                            start=False,
                            stop=False,
                            perf_mode=DR,
                        )

            # Part B: last 2 chunk groups col-tile-major, so state banks
            # close staggered (~850ns apart) and each col-tile's epilogue
            # overlaps the remaining matmuls instead of piling up at the end.
            # Outputs are merged per col-tile PAIR (one DMA per 2 tiles) so
            # the tail doesn't serialize on per-DMA HWDGE overhead.
            pair = None
            for c in range(CT):
                last = nb == NBLK - 1 and c == CT - 1
                hsplit = 2 if last else 1
                for hh in range(hsplit):
                    hr = range(2) if hsplit == 1 else [hh]
                    for q in range(QA, QS):
                        g, qq = divmod(q, SG)
                        lhs = wslice(wr_sb[g], 2 * qq, c)
                        for h in hr:
                            nc.tensor.matmul(
                                bank[(nb, c)][:, h * NH : (h + 1) * NH],
                                lhs,
                                mslice(s_ts[g], qq, h),
                                start=False,
                                stop=(q == QS - 1 and h == 1),
                                perf_mode=DR,
                            )
                tail = nb == NBLK - 1 and c >= CT - 2
                if pair is None:
                    pair = nspool.tile(
                        [P, 2 * NB], f16, tag="ns", name=f"ns_{nb}_{c}"
                    )
                po = (c % 2) * NB
                for hh in range(hsplit):
                    nq = 1
                    for qq2 in range(nq):
                        w = NB // (hsplit * nq)
                        lo = (hh * nq + qq2) * w
                        hs = slice(lo, lo + w)
                        tt = ttpool.tile([P, w], f16, tag="tt")
                        nc.scalar.activation(
                            tt[:], bank[(nb, c)][:, hs], AF.Tanh, scale=DESCALE
                        )
                        vv = vvpool.tile([P, w], f16, tag="vv")
                        nc.vector.tensor_tensor(
                            vv[:], tt[:], g01[(nb, c)][:, hs], ALU.mult
                        )
                        nc.vector.tensor_tensor(
                            pair[:, po + hs.start : po + hs.stop],
                            uu[(nb, c)][:, hs],
                            vv[:],
                            ALU.add,
                        )
                        if tail:
                            # Final two col tiles ship piecewise; the very
                            # last transfer is only 32 KB, minimizing the
                            # post-kernel DMA+semaphore tail.
                            nc.sync.dma_start(
                                ns_out[c * P : (c + 1) * P, bs][:, hs],
                                pair[:, po + hs.start : po + hs.stop],
                            )
                if c % 2 == 1:
                    if not tail:
                        nc.sync.dma_start(
                            ns_out[(c - 1) * P : (c + 1) * P, bs].rearrange(
                                "(c p) f -> p c f", p=P
                            ),
                            pair[:].rearrange("p (c f) -> p c f", c=2),
                        )
                    pair = None

    nc.compile()
    return nc


def _get_program():
    if "prog" not in _CACHE:
        _CACHE["prog"] = _build()
    return _CACHE["prog"]


def kernel(inputs, prev_output, reservoir_state, input_weights, reservoir_weights,
           gate_weights):
    import concourse.mybir as mybir
    from concourse.bass_utils import run_bass_kernel_spmd

    nc = _get_program()
    fp8 = mybir.dt.np(mybir.dt.float8e4)

    x = np.ascontiguousarray(np.asarray(inputs, dtype=np.float32))
    s = np.ascontiguousarray(np.asarray(reservoir_state, dtype=np.float32))
    w_in = np.ascontiguousarray(np.asarray(input_weights, dtype=np.float32))
    w_res = np.ascontiguousarray(np.asarray(reservoir_weights, dtype=np.float32))
    w_gate = np.ascontiguousarray(np.asarray(gate_weights, dtype=np.float32))

    xT = x.T  # [D_IN, B]
    sT = s.T  # [R, B]

    def pack_moving(aT):
        # [K, Bh] f32 -> fp8 [K/2, 2*Bh]; row q*128+p,
        # free nb*1024 + jj*512 + i  (see kernel layout comment)
        K, Bh = aT.shape
        a8 = (aT * SCALE_X).astype(fp8)
        a8 = a8.reshape(K // 256, 2, P, Bh // NB, NB)
        return np.ascontiguousarray(
            a8.transpose(0, 2, 3, 1, 4).reshape(K // 2, 2 * Bh)
        )

    def pack_weight(w):
        # [K, COLS] f32 -> fp8 [128, K/128*COLS]; row p, free j*COLS + c
        K = w.shape[0]
        w8 = (w * SCALE_W).astype(fp8)
        return np.ascontiguousarray(
            w8.reshape(K // P, P, COLS).transpose(1, 0, 2).reshape(P, -1)
        )

    # Batch-half moving operands (shared by the 4 column-group cores of
    # each half) and per-column-group weights (shared by both halves).
    x8s = [pack_moving(xT[:, m * B_CORE : (m + 1) * B_CORE]) for m in range(N_BG)]
    s8s = [pack_moving(sT[:, m * B_CORE : (m + 1) * B_CORE]) for m in range(N_BG)]
    wi8s, wr8s, wg8s = [], [], []
    for cg in range(N_CG):
        cslice = slice(cg * COLS, (cg + 1) * COLS)
        wi8s.append(pack_weight(w_in[:, cslice]))
        wr8s.append(pack_weight(w_res[:, cslice]))
        wg8s.append(pack_weight(w_gate[:, cslice]))

    in_maps = []
    for core in range(N_CORES):
        m, cg = divmod(core, N_CG)
        cslice = slice(cg * COLS, (cg + 1) * COLS)
        bslice = slice(m * B_CORE, (m + 1) * B_CORE)
        in_maps.append(
            {
                "x8": x8s[m],
                "s8": s8s[m],
                "w_in8": wi8s[cg],
                "w_res8": wr8s[cg],
                "w_gate8": wg8s[cg],
                "sepi": np.ascontiguousarray(
                    (0.9 * sT[cslice, bslice]).astype(np.float16)
                ),
            }
        )

    res = run_bass_kernel_spmd(nc, in_maps, list(range(N_CORES)))

    ns_T = np.empty((R, B), dtype=np.float16)
    for core in range(N_CORES):
        m, cg = divmod(core, N_CG)
        ns_T[cg * COLS : (cg + 1) * COLS, m * B_CORE : (m + 1) * B_CORE] = (
            np.asarray(res.results[core]["ns_out"])
        )
    new_state = np.ascontiguousarray(ns_T.T.astype(np.float32))  # [B, R]
    output = (new_state > 0.5).astype(np.float32)

    # fp8 matmuls carry ~4e-3 RMS noise into new_state; elements within
    # BAND of the spike threshold are re-evaluated exactly (f64 accumulation
    # over the true f32 inputs) and both outputs patched.
    bi, rj = np.nonzero(np.abs(new_state - 0.5) < BAND)
    for lo in range(0, bi.size, 32768):
        bc = bi[lo : lo + 32768]
        rc = rj[lo : lo + 32768]
        xg = x[bc]
        acc = np.einsum("ij,ji->i", xg, w_in[:, rc], dtype=np.float64,
                        optimize=True)
        acc += np.einsum("ij,ji->i", s[bc], w_res[:, rc], dtype=np.float64,
                         optimize=True)
        g_acc = np.einsum("ij,ji->i", xg, w_gate[:, rc], dtype=np.float64,
                          optimize=True)
        gate_v = 1.0 / (1.0 + np.exp(-g_acc))
        ns_fix = (0.9 * s[bc, rc].astype(np.float64)
                  + 0.1 * np.tanh(acc)) * gate_v
        ns_fix32 = ns_fix.astype(np.float32)
        new_state[bc, rc] = ns_fix32
        output[bc, rc] = (ns_fix32 > 0.5).astype(np.float32)
    return output, new_state


# revision 41
# speedup vs baseline: 3.6173x; 1.0061x over previous
"""Gated spiking reservoir step — Trainium2 Bass kernel (8 NeuronCores).

Math (per reference):
    ic   = inputs @ input_weights                  # [B, R]
    rc   = reservoir_state @ reservoir_weights     # [B, R]
    gate = sigmoid(inputs @ gate_weights)          # [B, R]
    ns   = (0.9 * reservoir_state + 0.1 * tanh(ic + rc)) * gate
    out  = (ns > 0.5) ? 1.0 : 0.0
    returns (out, ns)

Sharding: hybrid tensor/data parallel — 4 column groups x 2 batch halves.
Core (m, cg) owns a 1024-column slice of the three weight matrices and a
1024-row batch half, producing the [1024-col x 1024-batch] tile of both
outputs.  This minimizes per-core HBM traffic (~15 MB: weights 6 +
activations 5 + epilogue-state 2 + output 2).

All three GEMMs run in fp8 (e4m3) with the PE's DoubleRow perf mode: one
matmul instruction contracts 256 rows (2 fp8 rows per partition) at 0.5
cycles/row — 4x the fp32r rate.  Operands are pre-scaled on host (x*16,
s*16, w*512) so fp8 quantization lands in e4m3's normal range; the 1/8192
descale folds into the tanh/sigmoid activation scale.

The batch half is processed in 2 blocks of 512.  PSUM: 8 banks, one per
column tile of 128, in a 3-generation rotation: the gate GEMMs of BOTH
blocks run first (each group closed early, its sigmoid drains the bank),
then each block's state GEMM (input + reservoir) opens a fresh
accumulation group on the same bank.  All input DMAs are issued upfront
in consumption order (everything is SBUF-resident, ~14 MB), so the
schedule is paced purely by HBM bandwidth and PE throughput.

Epilogue per column tile: sigmoid/tanh on ScalarE (from PSUM, descaled),
then fast 2x-mode fp16 VectorE ops: u = (0.9s)*g and g01 = 0.1*g are
issued right after the sigmoid (off the critical path; host pre-scales
sepi by 0.9), leaving only m = tanh*g01 and ns = u + m after the last
matmul.  The reservoir GEMM's last 2 chunk groups run col-tile-major so
the per-tile epilogues overlap the remaining matmuls; fp16 outputs ship
as one transposed DMA per col-tile pair, piecewise for the final tile.

Host side: assemble/transpose, spike threshold, and an exact float64
re-evaluation of elements within BAND of the 0.5 threshold (fp8 matmul
noise ~4e-3 RMS would otherwise flip spikes near the boundary).
"""

import sys

if "/opt/trn_rl_repo" not in sys.path:
    sys.path.insert(0, "/opt/trn_rl_repo")

import numpy as np

B, D_IN, R = 2048, 1024, 4096
N_CORES = 8
N_CG = 4                     # column groups
N_BG = 2                     # batch groups
COLS = R // N_CG             # 1024 output columns per core
B_CORE = B // N_BG           # 1024 batch rows per core
P = 128                      # SBUF/PSUM partitions
CT = COLS // P               # 8 column tiles per core
NB = 512                     # batch block (one PSUM bank of f32)
NH = NB // 2                 # moving free dim per DoubleRow matmul
NBLK = B_CORE // NB          # 2 batch blocks
QX = D_IN // 256             # 4 double-row contraction tiles over input dim
QS = R // 256                # 16 double-row contraction tiles over reservoir
SG = 4                       # s8/w_res chunk granularity: 4 q-tiles

SCALE_X = 16.0               # fp8 pre-scale for activations (x and s)
SCALE_W = 512.0              # fp8 pre-scale for weights
DESCALE = 1.0 / (SCALE_X * SCALE_W)

# Re-evaluate elements this close to the 0.5 spike threshold exactly on
# host (fp8 matmul noise sigma ~4e-3; 0.018 ~ 4.5 sigma).
BAND = 0.018

_CACHE = {}


def _build():
    from contextlib import ExitStack

    from concourse import bacc, tile
    import concourse.mybir as mybir

    f32 = mybir.dt.float32
    f16 = mybir.dt.float16
    fp8 = mybir.dt.float8e4
    AF = mybir.ActivationFunctionType
    ALU = mybir.AluOpType
    DR = mybir.MatmulPerfMode.DoubleRow

    nc = bacc.Bacc(
        "TRN2", target_bir_lowering=False, debug=False, enable_asserts=False
    )

    # DRAM layouts (host pre-packed so every DMA is contiguous >=512B/row):
    #   x8   [512, 2048]  row q*128+p, free nb*1024 + jj*512 + i
    #                     = fp8(16 * x.T)[q*256+jj*128+p, bg*1024 + nb*512+i]
    #   s8   [2048, 2048] same layout over the 4096 reservoir rows
    #   w_*8 [128, J*COLS] row p, free j*COLS + c = fp8(512*w)[j*128+p, c]
    #   sepi [1024, 1024] fp16 (0.9 * s.T)[cols, batch-half]
    #   ns   [1024, 1024] fp16 output, same [col, batch] layout
    x8 = nc.dram_tensor("x8", [4 * P, NBLK * 1024], fp8, kind="ExternalInput")
    s8 = nc.dram_tensor("s8", [16 * P, NBLK * 1024], fp8, kind="ExternalInput")
    w_in8 = nc.dram_tensor("w_in8", [P, 8 * COLS], fp8, kind="ExternalInput")
    w_res8 = nc.dram_tensor("w_res8", [P, 32 * COLS], fp8, kind="ExternalInput")
    w_gate8 = nc.dram_tensor("w_gate8", [P, 8 * COLS], fp8, kind="ExternalInput")
    sepi = nc.dram_tensor("sepi", [COLS, B_CORE], f16, kind="ExternalInput")
    ns_out = nc.dram_tensor("ns_out", [COLS, B_CORE], f16, kind="ExternalOutput")

    def wslice(t, j2, ct):
        # [128, 2, 128] stationary view: rows {j2, j2+1}, cols ct*128..+128
        return t[:, j2 * COLS : (j2 + 2) * COLS].rearrange(
            "p (j c) -> p j c", j=2
        )[:, :, ct * P : (ct + 1) * P]

    def mslice(t, q, h):
        # [128, 2, 256] moving view from a flat [128, q*1024] block tile
        return t[:, q * 1024 : (q + 1) * 1024].rearrange(
            "p (j i) -> p j i", j=2
        )[:, :, h * NH : (h + 1) * NH]

    with tile.TileContext(nc) as tc, ExitStack() as ctx:
        wpool = ctx.enter_context(tc.tile_pool(name="weights", bufs=1))
        wg_sb = wpool.tile([P, 8 * COLS], fp8, tag="wg", name="wg_sb")
        wi_sb = wpool.tile([P, 8 * COLS], fp8, tag="wi", name="wi_sb")
        wr_sb = [
            wpool.tile([P, 2 * SG * COLS], fp8, tag=f"wr{g}", name=f"wr_sb{g}")
            for g in range(QS // SG)
        ]

        xpool = ctx.enter_context(tc.tile_pool(name="x_mov", bufs=NBLK))
        spool = ctx.enter_context(tc.tile_pool(name="s_mov", bufs=QS // SG * NBLK))
        sepool = ctx.enter_context(tc.tile_pool(name="s_epi", bufs=NBLK))
        banks = ctx.enter_context(tc.tile_pool(name="banks", bufs=CT, space="PSUM"))
        ggpool = ctx.enter_context(tc.tile_pool(name="gg", bufs=2 * CT))
        g1pool = ctx.enter_context(tc.tile_pool(name="gg01", bufs=2 * CT))
        uupool = ctx.enter_context(tc.tile_pool(name="uu", bufs=2 * CT))
        ttpool = ctx.enter_context(tc.tile_pool(name="tt", bufs=CT))
        vvpool = ctx.enter_context(tc.tile_pool(name="vv", bufs=CT))
        nspool = ctx.enter_context(tc.tile_pool(name="ns", bufs=2 * CT))

        # Everything is SBUF-resident (~14 MB): issue ALL input DMAs upfront
        # in consumption order (the first matmuls need half of w_gate plus
        # block 0's x, so those two lead), and the SP queue never head-of-line
        # blocks a load behind an output store that waits on compute.
        xts, seps, s_tss = [], [], []

        def load_x(nb, half=None):
            if half is None:
                xt = xpool.tile([P, QX * 1024], fp8, tag="x", name=f"x_{nb}")
                xts.append(xt)
                qr = slice(0, QX)
            else:
                if half == 0:
                    xt = xpool.tile([P, QX * 1024], fp8, tag="x",
                                    name=f"x_{nb}")
                    xts.append(xt)
                xt = xts[nb]
                qr = slice(half * QX // 2, (half + 1) * QX // 2)
            nc.sync.dma_start(
                xt[:].rearrange("p (q f) -> p q f", q=QX)[:, qr],
                x8[:, nb * 1024 : (nb + 1) * 1024].rearrange(
                    "(q p) f -> p q f", p=P
                )[:, qr],
            )

        def load_wg_half(k):
            hw = COLS // 2
            nc.sync.dma_start(
                wg_sb[:].rearrange("p (j c) -> p j c", j=8)
                [:, :, k * hw : (k + 1) * hw],
                w_gate8[:, :].rearrange("p (j c) -> p j c", j=8)
                [:, :, k * hw : (k + 1) * hw],
            )

        load_wg_half(0)
        load_x(0, 0)
        load_x(0, 1)
        load_wg_half(1)
        load_x(1)
        nc.sync.dma_start(wi_sb[:], w_in8[:, :])
        for nb in range(NBLK):
            bs1k = slice(nb * 1024, (nb + 1) * 1024)
            bs = slice(nb * NB, (nb + 1) * NB)
            s_ts = []
            for g in range(QS // SG):
                if nb == 0:
                    nc.sync.dma_start(
                        wr_sb[g][:],
                        w_res8[:, g * 2 * SG * COLS : (g + 1) * 2 * SG * COLS],
                    )
                st = spool.tile([P, SG * 1024], fp8, tag="s8", name=f"s8_{nb}_{g}")
                nc.sync.dma_start(
                    st[:].rearrange("p (q f) -> p q f", q=SG),
                    s8[g * SG * P : (g + 1) * SG * P, bs1k].rearrange(
                        "(q p) f -> p q f", p=P
                    ),
                )
                s_ts.append(st)
            s_tss.append(s_ts)
            sep = sepool.tile([P, CT * NB], f16, tag="sepi", name=f"sepi_{nb}")
            nc.sync.dma_start(
                sep[:].rearrange("p (c f) -> p c f", c=CT),
                sepi[:, bs].rearrange("(c p) f -> p c f", p=P),
            )
            seps.append(sep)

        # Gate GEMMs for BOTH blocks run first: each bank's gate group closes
        # early and its sigmoid drains it while later gate tiles accumulate.
        # The state phases then allocate fresh tiles from the same pool — a
        # 3-generation buffer rotation (gate nb0/nb1, state nb0, state nb1)
        # whose write-after-read waits (sigmoid / previous tanh) all retire
        # well before the new group's first matmul.
        bank, gg, uu, g01 = {}, {}, {}, {}
        for nb in range(NBLK):
            xt, sep = xts[nb], seps[nb]
            for c in range(CT):
                bk = banks.tile([P, NB], f32, tag="bank", name=f"bankG_{nb}_{c}")
                bank[(nb, c)] = bk
                for q in range(QX):
                    lhs = wslice(wg_sb, 2 * q, c)
                    for h in range(2):
                        nc.tensor.matmul(
                            bk[:, h * NH : (h + 1) * NH],
                            lhs,
                            mslice(xt, q, h),
                            start=(q == 0 and h == 0),
                            stop=(q == QX - 1 and h == 1),
                            perf_mode=DR,
                        )
                g = ggpool.tile([P, NB], f16, tag="gg", name=f"gg_{nb}_{c}")
                nc.scalar.activation(g[:], bk[:], AF.Sigmoid, scale=DESCALE)
                gg[(nb, c)] = g
                # Gate-dependent products that do NOT need tanh: u = 0.9*s*g
                # and g01 = 0.1*g; off the critical path, fast 2x fp16 DVE.
                cs = slice(c * NB, (c + 1) * NB)
                u = uupool.tile([P, NB], f16, tag="uu", name=f"uu_{nb}_{c}")
                nc.vector.tensor_tensor(u[:], sep[:, cs], g[:], ALU.mult)
                uu[(nb, c)] = u
                g1 = g1pool.tile([P, NB], f16, tag="gg01", name=f"g01_{nb}_{c}")
                nc.vector.tensor_scalar(g1[:], g[:], 0.1, None, ALU.mult)
                g01[(nb, c)] = g1

        for nb in range(NBLK):
            QA = QS - 2 * SG if nb < NBLK - 1 else QS - 3 * SG
            bs = slice(nb * NB, (nb + 1) * NB)
            xt, s_ts = xts[nb], s_tss[nb]
            for c in range(CT):
                bank[(nb, c)] = banks.tile(
                    [P, NB], f32, tag="bank", name=f"bankS_{nb}_{c}"
                )

            # Input GEMM opens the state accumulation groups.  Col-tile-major
            # so col tile c7 (whose bank is freed last by the previous
            # block's tanh) is touched ~3us later than c0.
            for c in range(CT):
                for q in range(QX):
                    lhs = wslice(wi_sb, 2 * q, c)
                    for h in range(2):
                        nc.tensor.matmul(
                            bank[(nb, c)][:, h * NH : (h + 1) * NH],
                            lhs,
                            mslice(xt, q, h),
                            start=(q == 0 and h == 0),
                            stop=False,
                            perf_mode=DR,
                        )

            # Reservoir GEMM part A: q-major over the first 2 chunk groups so
            # consumption tracks the s8/w_res DMA stream.
            for q in range(QA):
                g, qq = divmod(q, SG)
                for c in range(CT):
                    lhs = wslice(wr_sb[g], 2 * qq, c)
                    for h in range(2):
                        nc.tensor.matmul(
                            bank[(nb, c)][:, h * NH : (h + 1) * NH],
                            lhs,
                            mslice(s_ts[g], qq, h),
                            start=False,
                            stop=False,
                            perf_mode=DR,
                        )

            # Part B: last 2 chunk groups col-tile-major, so state banks
            # close staggered (~850ns apart) and each col-tile's epilogue
            # overlaps the remaining matmuls instead of piling up at the end.
            # Outputs are merged per col-tile PAIR (one DMA per 2 tiles) so
            # the tail doesn't serialize on per-DMA HWDGE overhead.
            pair = None
            for c in range(CT):
                last = nb == NBLK - 1 and c == CT - 1
                hsplit = 2 if last else 1
                for hh in range(hsplit):
                    hr = range(2) if hsplit == 1 else [hh]
                    for q in range(QA, QS):
                        g, qq = divmod(q, SG)
                        lhs = wslice(wr_sb[g], 2 * qq, c)
                        for h in hr:
                            nc.tensor.matmul(
                                bank[(nb, c)][:, h * NH : (h + 1) * NH],
                                lhs,
                                mslice(s_ts[g], qq, h),
                                start=False,
                                stop=(q == QS - 1 and h == 1),
                                perf_mode=DR,
                            )
                tail = nb == NBLK - 1 and c >= CT - 2
                if pair is None:
                    pair = nspool.tile(
                        [P, 2 * NB], f16, tag="ns", name=f"ns_{nb}_{c}"
                    )
                po = (c % 2) * NB
                for hh in range(hsplit):
                    nq = 1
                    for qq2 in range(nq):
                        w = NB // (hsplit * nq)
                        lo = (hh * nq + qq2) * w
                        hs = slice(lo, lo + w)
                        tt = ttpool.tile([P, w], f16, tag="tt")
                        nc.scalar.activation(
                            tt[:], bank[(nb, c)][:, hs], AF.Tanh, scale=DESCALE
                        )
                        vv = vvpool.tile([P, w], f16, tag="vv")
                        nc.vector.tensor_tensor(
                            vv[:], tt[:], g01[(nb, c)][:, hs], ALU.mult
                        )
                        nc.vector.tensor_tensor(
                            pair[:, po + hs.start : po + hs.stop],
                            uu[(nb, c)][:, hs],
                            vv[:],
                            ALU.add,
                        )
                        if tail:
                            # Final two col tiles ship piecewise; the very
                            # last transfer is only 32 KB, minimizing the
                            # post-kernel DMA+semaphore tail.
                            nc.sync.dma_start(
                                ns_out[c * P : (c + 1) * P, bs][:, hs],
                                pair[:, po + hs.start : po + hs.stop],
                            )
                if c % 2 == 1:
                    if not tail:
                        nc.sync.dma_start(
                            ns_out[(c - 1) * P : (c + 1) * P, bs].rearrange(
                                "(c p) f -> p c f", p=P
                            ),
                            pair[:].rearrange("p (c f) -> p c f", c=2),
                        )
                    pair = None

    nc.compile()
    return nc


def _get_program():
    if "prog" not in _CACHE:
        _CACHE["prog"] = _build()
    return _CACHE["prog"]


def kernel(inputs, prev_output, reservoir_state, input_weights, reservoir_weights,
           gate_weights):
    import concourse.mybir as mybir
    from concourse.bass_utils import run_bass_kernel_spmd

    nc = _get_program()
    fp8 = mybir.dt.np(mybir.dt.float8e4)

    x = np.ascontiguousarray(np.asarray(inputs, dtype=np.float32))
    s = np.ascontiguousarray(np.asarray(reservoir_state, dtype=np.float32))
    w_in = np.ascontiguousarray(np.asarray(input_weights, dtype=np.float32))
    w_res = np.ascontiguousarray(np.asarray(reservoir_weights, dtype=np.float32))
    w_gate = np.ascontiguousarray(np.asarray(gate_weights, dtype=np.float32))

    xT = x.T  # [D_IN, B]
    sT = s.T  # [R, B]

    def pack_moving(aT):
        # [K, Bh] f32 -> fp8 [K/2, 2*Bh]; row q*128+p,
        # free nb*1024 + jj*512 + i  (see kernel layout comment)
        K, Bh = aT.shape
        a8 = (aT * SCALE_X).astype(fp8)
        a8 = a8.reshape(K // 256, 2, P, Bh // NB, NB)
        return np.ascontiguousarray(
            a8.transpose(0, 2, 3, 1, 4).reshape(K // 2, 2 * Bh)
        )

    def pack_weight(w):
        # [K, COLS] f32 -> fp8 [128, K/128*COLS]; row p, free j*COLS + c
        K = w.shape[0]
        w8 = (w * SCALE_W).astype(fp8)
        return np.ascontiguousarray(
            w8.reshape(K // P, P, COLS).transpose(1, 0, 2).reshape(P, -1)
        )

    # Batch-half moving operands (shared by the 4 column-group cores of
    # each half) and per-column-group weights (shared by both halves).
    x8s = [pack_moving(xT[:, m * B_CORE : (m + 1) * B_CORE]) for m in range(N_BG)]
    s8s = [pack_moving(sT[:, m * B_CORE : (m + 1) * B_CORE]) for m in range(N_BG)]
    wi8s, wr8s, wg8s = [], [], []
    for cg in range(N_CG):
        cslice = slice(cg * COLS, (cg + 1) * COLS)
        wi8s.append(pack_weight(w_in[:, cslice]))
        wr8s.append(pack_weight(w_res[:, cslice]))
        wg8s.append(pack_weight(w_gate[:, cslice]))

    in_maps = []
    for core in range(N_CORES):
        m, cg = divmod(core, N_CG)
        cslice = slice(cg * COLS, (cg + 1) * COLS)
        bslice = slice(m * B_CORE, (m + 1) * B_CORE)
        in_maps.append(
            {
                "x8": x8s[m],
                "s8": s8s[m],
                "w_in8": wi8s[cg],
                "w_res8": wr8s[cg],
                "w_gate8": wg8s[cg],
                "sepi": np.ascontiguousarray(
                    (0.9 * sT[cslice, bslice]).astype(np.float16)
                ),
            }
        )

    res = run_bass_kernel_spmd(nc, in_maps, list(range(N_CORES)))

    ns_T = np.empty((R, B), dtype=np.float16)
    for core in range(N_CORES):
        m, cg = divmod(core, N_CG)
        ns_T[cg * COLS : (cg + 1) * COLS, m * B_CORE : (m + 1) * B_CORE] = (
            np.asarray(res.results[core]["ns_out"])
        )
    new_state = np.ascontiguousarray(ns_T.T.astype(np.float32))  # [B, R]
    output = (new_state > 0.5).astype(np.float32)

    # fp8 matmuls carry ~4e-3 RMS noise into new_state; elements within
    # BAND of the spike threshold are re-evaluated exactly (f64 accumulation
    # over the true f32 inputs) and both outputs patched.
    bi, rj = np.nonzero(np.abs(new_state - 0.5) < BAND)
    for lo in range(0, bi.size, 32768):
        bc = bi[lo : lo + 32768]
        rc = rj[lo : lo + 32768]
        xg = x[bc]
        acc = np.einsum("ij,ji->i", xg, w_in[:, rc], dtype=np.float64,
                        optimize=True)
        acc += np.einsum("ij,ji->i", s[bc], w_res[:, rc], dtype=np.float64,
                         optimize=True)
        g_acc = np.einsum("ij,ji->i", xg, w_gate[:, rc], dtype=np.float64,
                          optimize=True)
        gate_v = 1.0 / (1.0 + np.exp(-g_acc))
        ns_fix = (0.9 * s[bc, rc].astype(np.float64)
                  + 0.1 * np.tanh(acc)) * gate_v
        ns_fix32 = ns_fix.astype(np.float32)
        new_state[bc, rc] = ns_fix32
        output[bc, rc] = (ns_fix32 > 0.5).astype(np.float32)
    return output, new_state


# revision 47
# speedup vs baseline: 3.6419x; 1.0068x over previous
"""Gated spiking reservoir step — Trainium2 Bass kernel (8 NeuronCores).

Math (per reference):
    ic   = inputs @ input_weights                  # [B, R]
    rc   = reservoir_state @ reservoir_weights     # [B, R]
    gate = sigmoid(inputs @ gate_weights)          # [B, R]
    ns   = (0.9 * reservoir_state + 0.1 * tanh(ic + rc)) * gate
    out  = (ns > 0.5) ? 1.0 : 0.0
    returns (out, ns)

Sharding: hybrid tensor/data parallel — 4 column groups x 2 batch halves.
Core (m, cg) owns a 1024-column slice of the three weight matrices and a
1024-row batch half, producing the [1024-col x 1024-batch] tile of both
outputs.  This minimizes per-core HBM traffic (~15 MB: weights 6 +
activations 5 + epilogue-state 2 + output 2).

All three GEMMs run in fp8 (e4m3) with the PE's DoubleRow perf mode: one
matmul instruction contracts 256 rows (2 fp8 rows per partition) at 0.5
cycles/row — 4x the fp32r rate.  Operands are pre-scaled on host (x*16,
s*16, w*512) so fp8 quantization lands in e4m3's normal range; the 1/8192
descale folds into the tanh/sigmoid activation scale.

The batch half is processed in 2 blocks of 512.  PSUM: 8 banks, one per
column tile of 128, in a 3-generation rotation: the gate GEMMs of BOTH
blocks run first (each group closed early, its sigmoid drains the bank),
then each block's state GEMM (input + reservoir) opens a fresh
accumulation group on the same bank.  All input DMAs are issued upfront
in consumption order (everything is SBUF-resident, ~14 MB), so the
schedule is paced purely by HBM bandwidth and PE throughput.

Epilogue per column tile: sigmoid/tanh on ScalarE (from PSUM, descaled),
then fast 2x-mode fp16 VectorE ops: u = (0.9s)*g and g01 = 0.1*g are
issued right after the sigmoid (off the critical path; host pre-scales
sepi by 0.9), leaving only m = tanh*g01 and ns = u + m after the last
matmul.  The reservoir GEMM's last 2 chunk groups run col-tile-major so
the per-tile epilogues overlap the remaining matmuls; fp16 outputs ship
as one transposed DMA per col-tile pair, piecewise for the final tile.

Host side: assemble/transpose, spike threshold, and an exact float64
re-evaluation of elements within BAND of the 0.5 threshold (fp8 matmul
noise ~4e-3 RMS would otherwise flip spikes near the boundary).
"""

import sys

if "/opt/trn_rl_repo" not in sys.path:
    sys.path.insert(0, "/opt/trn_rl_repo")

import numpy as np

B, D_IN, R = 2048, 1024, 4096
N_CORES = 8
N_CG = 4                     # column groups
N_BG = 2                     # batch groups
COLS = R // N_CG             # 1024 output columns per core
B_CORE = B // N_BG           # 1024 batch rows per core
P = 128                      # SBUF/PSUM partitions
CT = COLS // P               # 8 column tiles per core
NB = 512                     # batch block (one PSUM bank of f32)
NH = NB // 2                 # moving free dim per DoubleRow matmul
NBLK = B_CORE // NB          # 2 batch blocks
QX = D_IN // 256             # 4 double-row contraction tiles over input dim
QS = R // 256                # 16 double-row contraction tiles over reservoir
SG = 4                       # s8/w_res chunk granularity: 4 q-tiles

SCALE_X = 16.0               # fp8 pre-scale for activations (x and s)
SCALE_W = 512.0              # fp8 pre-scale for weights
DESCALE = 1.0 / (SCALE_X * SCALE_W)

# Re-evaluate elements this close to the 0.5 spike threshold exactly on
# host (fp8 matmul noise sigma ~4e-3; 0.018 ~ 4.5 sigma).
BAND = 0.018

_CACHE = {}


def _build():
    from contextlib import ExitStack

    from concourse import bacc, tile
    import concourse.mybir as mybir

    f32 = mybir.dt.float32
    f16 = mybir.dt.float16
    fp8 = mybir.dt.float8e4
    AF = mybir.ActivationFunctionType
    ALU = mybir.AluOpType
    DR = mybir.MatmulPerfMode.DoubleRow

    nc = bacc.Bacc(
        "TRN2", target_bir_lowering=False, debug=False, enable_asserts=False
    )

    # DRAM layouts (host pre-packed so every DMA is contiguous >=512B/row):
    #   x8   [512, 2048]  row q*128+p, free nb*1024 + jj*512 + i
    #                     = fp8(16 * x.T)[q*256+jj*128+p, bg*1024 + nb*512+i]
    #   s8   [2048, 2048] same layout over the 4096 reservoir rows
    #   w_*8 [128, J*COLS] row p, free j*COLS + c = fp8(512*w)[j*128+p, c]
    #   sepi [1024, 1024] fp16 (0.9 * s.T)[cols, batch-half]
    #   ns   [1024, 1024] fp16 output, same [col, batch] layout
    x8 = nc.dram_tensor("x8", [4 * P, NBLK * 1024], fp8, kind="ExternalInput")
    s8 = nc.dram_tensor("s8", [16 * P, NBLK * 1024], fp8, kind="ExternalInput")
    w_in8 = nc.dram_tensor("w_in8", [P, 8 * COLS], fp8, kind="ExternalInput")
    w_res8 = nc.dram_tensor("w_res8", [P, 32 * COLS], fp8, kind="ExternalInput")
    w_gate8 = nc.dram_tensor("w_gate8", [P, 8 * COLS], fp8, kind="ExternalInput")
    sepi = nc.dram_tensor("sepi", [COLS, B_CORE], f16, kind="ExternalInput")
    ns_out = nc.dram_tensor("ns_out", [COLS, B_CORE], f16, kind="ExternalOutput")

    def wslice(t, j2, ct):
        # [128, 2, 128] stationary view: rows {j2, j2+1}, cols ct*128..+128
        return t[:, j2 * COLS : (j2 + 2) * COLS].rearrange(
            "p (j c) -> p j c", j=2
        )[:, :, ct * P : (ct + 1) * P]

    def mslice(t, q, h):
        # [128, 2, 256] moving view from a flat [128, q*1024] block tile
        return t[:, q * 1024 : (q + 1) * 1024].rearrange(
            "p (j i) -> p j i", j=2
        )[:, :, h * NH : (h + 1) * NH]

    with tile.TileContext(nc) as tc, ExitStack() as ctx:
        wpool = ctx.enter_context(tc.tile_pool(name="weights", bufs=1))
        wg_sb = wpool.tile([P, 8 * COLS], fp8, tag="wg", name="wg_sb")
        wi_sb = wpool.tile([P, 8 * COLS], fp8, tag="wi", name="wi_sb")
        wr_sb = [
            wpool.tile([P, 2 * SG * COLS], fp8, tag=f"wr{g}", name=f"wr_sb{g}")
            for g in range(QS // SG)
        ]

        xpool = ctx.enter_context(tc.tile_pool(name="x_mov", bufs=NBLK))
        spool = ctx.enter_context(tc.tile_pool(name="s_mov", bufs=QS // SG * NBLK))
        sepool = ctx.enter_context(tc.tile_pool(name="s_epi", bufs=NBLK))
        banks = ctx.enter_context(tc.tile_pool(name="banks", bufs=CT, space="PSUM"))
        ggpool = ctx.enter_context(tc.tile_pool(name="gg", bufs=2 * CT))
        g1pool = ctx.enter_context(tc.tile_pool(name="gg01", bufs=2 * CT))
        uupool = ctx.enter_context(tc.tile_pool(name="uu", bufs=2 * CT))
        ttpool = ctx.enter_context(tc.tile_pool(name="tt", bufs=CT))
        vvpool = ctx.enter_context(tc.tile_pool(name="vv", bufs=CT))
        nspool = ctx.enter_context(tc.tile_pool(name="ns", bufs=2 * CT))

        # Everything is SBUF-resident (~14 MB): issue ALL input DMAs upfront
        # in consumption order (the first matmuls need half of w_gate plus
        # block 0's x, so those two lead), and the SP queue never head-of-line
        # blocks a load behind an output store that waits on compute.
        xts, seps, s_tss = [], [], []

        def load_x(nb, half=None):
            if half is None:
                xt = xpool.tile([P, QX * 1024], fp8, tag="x", name=f"x_{nb}")
                xts.append(xt)
                qr = slice(0, QX)
            else:
                if half == 0:
                    xt = xpool.tile([P, QX * 1024], fp8, tag="x",
                                    name=f"x_{nb}")
                    xts.append(xt)
                xt = xts[nb]
                qr = slice(half * QX // 2, (half + 1) * QX // 2)
            nc.sync.dma_start(
                xt[:].rearrange("p (q f) -> p q f", q=QX)[:, qr],
                x8[:, nb * 1024 : (nb + 1) * 1024].rearrange(
                    "(q p) f -> p q f", p=P
                )[:, qr],
            )

        def load_wg_half(k):
            hw = COLS // 2
            nc.sync.dma_start(
                wg_sb[:].rearrange("p (j c) -> p j c", j=8)
                [:, :, k * hw : (k + 1) * hw],
                w_gate8[:, :].rearrange("p (j c) -> p j c", j=8)
                [:, :, k * hw : (k + 1) * hw],
            )

        load_wg_half(0)
        load_x(0, 0)
        load_x(0, 1)
        load_wg_half(1)
        load_x(1)
        nc.sync.dma_start(wi_sb[:], w_in8[:, :])
        for nb in range(NBLK):
            bs1k = slice(nb * 1024, (nb + 1) * 1024)
            bs = slice(nb * NB, (nb + 1) * NB)
            s_ts = []
            for g in range(QS // SG):
                if nb == 0:
                    nc.sync.dma_start(
                        wr_sb[g][:],
                        w_res8[:, g * 2 * SG * COLS : (g + 1) * 2 * SG * COLS],
                    )
                st = spool.tile([P, SG * 1024], fp8, tag="s8", name=f"s8_{nb}_{g}")
                nc.sync.dma_start(
                    st[:].rearrange("p (q f) -> p q f", q=SG),
                    s8[g * SG * P : (g + 1) * SG * P, bs1k].rearrange(
                        "(q p) f -> p q f", p=P
                    ),
                )
                s_ts.append(st)
            s_tss.append(s_ts)
            sep = sepool.tile([P, CT * NB], f16, tag="sepi", name=f"sepi_{nb}")
            nc.sync.dma_start(
                sep[:].rearrange("p (c f) -> p c f", c=CT),
                sepi[:, bs].rearrange("(c p) f -> p c f", p=P),
            )
            seps.append(sep)

        # Gate GEMMs for BOTH blocks run first: each bank's gate group closes
        # early and its sigmoid drains it while later gate tiles accumulate.
        # The state phases then allocate fresh tiles from the same pool — a
        # 3-generation buffer rotation (gate nb0/nb1, state nb0, state nb1)
        # whose write-after-read waits (sigmoid / previous tanh) all retire
        # well before the new group's first matmul.
        bank, gg, uu, g01 = {}, {}, {}, {}
        for nb in range(NBLK):
            xt, sep = xts[nb], seps[nb]
            for c in range(CT):
                bk = banks.tile([P, NB], f32, tag="bank", name=f"bankG_{nb}_{c}")
                bank[(nb, c)] = bk
                for q in range(QX):
                    lhs = wslice(wg_sb, 2 * q, c)
                    for h in range(2):
                        nc.tensor.matmul(
                            bk[:, h * NH : (h + 1) * NH],
                            lhs,
                            mslice(xt, q, h),
                            start=(q == 0 and h == 0),
                            stop=(q == QX - 1 and h == 1),
                            perf_mode=DR,
                        )
                g = ggpool.tile([P, NB], f16, tag="gg", name=f"gg_{nb}_{c}")
                nc.scalar.activation(g[:], bk[:], AF.Sigmoid, scale=DESCALE)
                gg[(nb, c)] = g
                # Gate-dependent products that do NOT need tanh: u = 0.9*s*g
                # and g01 = 0.1*g; off the critical path, fast 2x fp16 DVE.
                cs = slice(c * NB, (c + 1) * NB)
                u = uupool.tile([P, NB], f16, tag="uu", name=f"uu_{nb}_{c}")
                nc.vector.tensor_tensor(u[:], sep[:, cs], g[:], ALU.mult)
                uu[(nb, c)] = u
                g1 = g1pool.tile([P, NB], f16, tag="gg01", name=f"g01_{nb}_{c}")
                nc.vector.tensor_scalar(g1[:], g[:], 0.1, None, ALU.mult)
                g01[(nb, c)] = g1

        for nb in range(NBLK):
            QA = QS - 2 * SG if nb < NBLK - 1 else 0
            bs = slice(nb * NB, (nb + 1) * NB)
            xt, s_ts = xts[nb], s_tss[nb]
            for c in range(CT):
                bank[(nb, c)] = banks.tile(
                    [P, NB], f32, tag="bank", name=f"bankS_{nb}_{c}"
                )

            # Input GEMM opens the state accumulation groups.  Col-tile-major
            # so col tile c7 (whose bank is freed last by the previous
            # block's tanh) is touched ~3us later than c0.  For the last
            # block it is fused into the per-col-tile sweep below instead.
            def in_mms(c, hr=range(2)):
                for q in range(QX):
                    lhs = wslice(wi_sb, 2 * q, c)
                    for h in hr:
                        nc.tensor.matmul(
                            bank[(nb, c)][:, h * NH : (h + 1) * NH],
                            lhs,
                            mslice(xt, q, h),
                            start=(q == 0 and h == (0 if 0 in hr else 1)),
                            stop=False,
                            perf_mode=DR,
                        )

            if nb < NBLK - 1:
                for c in range(CT):
                    in_mms(c)

            # Reservoir GEMM part A: q-major over the first 2 chunk groups so
            # consumption tracks the s8/w_res DMA stream.
            for q in range(QA):
                g, qq = divmod(q, SG)
                for c in range(CT):
                    lhs = wslice(wr_sb[g], 2 * qq, c)
                    for h in range(2):
                        nc.tensor.matmul(
                            bank[(nb, c)][:, h * NH : (h + 1) * NH],
                            lhs,
                            mslice(s_ts[g], qq, h),
                            start=False,
                            stop=False,
                            perf_mode=DR,
                        )

            # Part B: remaining chunk groups col-tile-major, so state banks
            # close staggered and each col-tile's epilogue overlaps the
            # remaining matmuls instead of piling up at the end.  Outputs are
            # merged per col-tile PAIR (one DMA per 2 tiles) so the tail
            # doesn't serialize on per-DMA HWDGE overhead.

            def res_mms(c, hr):
                for q in range(QA, QS):
                    g, qq = divmod(q, SG)
                    lhs = wslice(wr_sb[g], 2 * qq, c)
                    for h in hr:
                        nc.tensor.matmul(
                            bank[(nb, c)][:, h * NH : (h + 1) * NH],
                            lhs,
                            mslice(s_ts[g], qq, h),
                            start=False,
                            stop=(q == QS - 1 and h == 1),
                            perf_mode=DR,
                        )

            def epi(c, hs, pr, po, ship):
                w = hs.stop - hs.start
                tt = ttpool.tile([P, w], f16, tag="tt")
                nc.scalar.activation(
                    tt[:], bank[(nb, c)][:, hs], AF.Tanh, scale=DESCALE
                )
                vv = vvpool.tile([P, w], f16, tag="vv")
                nc.vector.tensor_tensor(
                    vv[:], tt[:], g01[(nb, c)][:, hs], ALU.mult
                )
                nc.vector.tensor_tensor(
                    pr[:, po + hs.start : po + hs.stop],
                    uu[(nb, c)][:, hs],
                    vv[:],
                    ALU.add,
                )
                if ship:
                    nc.sync.dma_start(
                        ns_out[c * P : (c + 1) * P, bs][:, hs],
                        pr[:, po + hs.start : po + hs.stop],
                    )

            pair = None
            ct_plain = CT if nb < NBLK - 1 else CT - 2
            for c in range(ct_plain):
                if nb == NBLK - 1:
                    in_mms(c)
                res_mms(c, range(2))
                if pair is None:
                    pair = nspool.tile(
                        [P, 2 * NB], f16, tag="ns", name=f"ns_{nb}_{c}"
                    )
                epi(c, slice(0, NB), pair, (c % 2) * NB, ship=False)
                if c % 2 == 1:
                    nc.sync.dma_start(
                        ns_out[(c - 1) * P : (c + 1) * P, bs].rearrange(
                            "(c p) f -> p c f", p=P
                        ),
                        pair[:].rearrange("p (c f) -> p c f", c=2),
                    )
                    pair = None

            if nb == NBLK - 1:
                # Final two col tiles, reordered as [c7-h0, c6, c7-h1] so
                # c7's first tanh half runs during c6's matmuls and only a
                # 256-wide tanh -> 2 DVE ops -> 64 KB DMA chain remains
                # after the very last matmul.
                c6, c7 = CT - 2, CT - 1
                pr = nspool.tile([P, 2 * NB], f16, tag="ns", name=f"ns_{nb}_t")
                in_mms(c7)
                res_mms(c7, [0])
                epi(c7, slice(0, NH), pr, NB, ship=True)
                in_mms(c6)
                res_mms(c6, range(2))
                epi(c6, slice(0, NB), pr, 0, ship=True)
                res_mms(c7, [1])
                epi(c7, slice(NH, NB), pr, NB, ship=True)

    nc.compile()
    return nc


def _get_program():
    if "prog" not in _CACHE:
        _CACHE["prog"] = _build()
    return _CACHE["prog"]


def kernel(inputs, prev_output, reservoir_state, input_weights, reservoir_weights,
           gate_weights):
    import concourse.mybir as mybir
    from concourse.bass_utils import run_bass_kernel_spmd

    nc = _get_program()
    fp8 = mybir.dt.np(mybir.dt.float8e4)

    x = np.ascontiguousarray(np.asarray(inputs, dtype=np.float32))
    s = np.ascontiguousarray(np.asarray(reservoir_state, dtype=np.float32))
    w_in = np.ascontiguousarray(np.asarray(input_weights, dtype=np.float32))
    w_res = np.ascontiguousarray(np.asarray(reservoir_weights, dtype=np.float32))
    w_gate = np.ascontiguousarray(np.asarray(gate_weights, dtype=np.float32))

    xT = x.T  # [D_IN, B]
    sT = s.T  # [R, B]

    def pack_moving(aT):
        # [K, Bh] f32 -> fp8 [K/2, 2*Bh]; row q*128+p,
        # free nb*1024 + jj*512 + i  (see kernel layout comment)
        K, Bh = aT.shape
        a8 = (aT * SCALE_X).astype(fp8)
        a8 = a8.reshape(K // 256, 2, P, Bh // NB, NB)
        return np.ascontiguousarray(
            a8.transpose(0, 2, 3, 1, 4).reshape(K // 2, 2 * Bh)
        )

    def pack_weight(w):
        # [K, COLS] f32 -> fp8 [128, K/128*COLS]; row p, free j*COLS + c
        K = w.shape[0]
        w8 = (w * SCALE_W).astype(fp8)
        return np.ascontiguousarray(
            w8.reshape(K // P, P, COLS).transpose(1, 0, 2).reshape(P, -1)
        )

    # Batch-half moving operands (shared by the 4 column-group cores of
    # each half) and per-column-group weights (shared by both halves).
    x8s = [pack_moving(xT[:, m * B_CORE : (m + 1) * B_CORE]) for m in range(N_BG)]
    s8s = [pack_moving(sT[:, m * B_CORE : (m + 1) * B_CORE]) for m in range(N_BG)]
    wi8s, wr8s, wg8s = [], [], []
    for cg in range(N_CG):
        cslice = slice(cg * COLS, (cg + 1) * COLS)
        wi8s.append(pack_weight(w_in[:, cslice]))
        wr8s.append(pack_weight(w_res[:, cslice]))
        wg8s.append(pack_weight(w_gate[:, cslice]))

    in_maps = []
    for core in range(N_CORES):
        m, cg = divmod(core, N_CG)
        cslice = slice(cg * COLS, (cg + 1) * COLS)
        bslice = slice(m * B_CORE, (m + 1) * B_CORE)
        in_maps.append(
            {
                "x8": x8s[m],
                "s8": s8s[m],
                "w_in8": wi8s[cg],
                "w_res8": wr8s[cg],
                "w_gate8": wg8s[cg],
                "sepi": np.ascontiguousarray(
                    (0.9 * sT[cslice, bslice]).astype(np.float16)
                ),
            }
        )

    res = run_bass_kernel_spmd(nc, in_maps, list(range(N_CORES)))

    ns_T = np.empty((R, B), dtype=np.float16)
    for core in range(N_CORES):
        m, cg = divmod(core, N_CG)
        ns_T[cg * COLS : (cg + 1) * COLS, m * B_CORE : (m + 1) * B_CORE] = (
            np.asarray(res.results[core]["ns_out"])
        )
    new_state = np.ascontiguousarray(ns_T.T.astype(np.float32))  # [B, R]
    output = (new_state > 0.5).astype(np.float32)

    # fp8 matmuls carry ~4e-3 RMS noise into new_state; elements within
    # BAND of the spike threshold are re-evaluated exactly (f64 accumulation
    # over the true f32 inputs) and both outputs patched.
    bi, rj = np.nonzero(np.abs(new_state - 0.5) < BAND)
    for lo in range(0, bi.size, 32768):
        bc = bi[lo : lo + 32768]
        rc = rj[lo : lo + 32768]
        xg = x[bc]
        acc = np.einsum("ij,ji->i", xg, w_in[:, rc], dtype=np.float64,
                        optimize=True)
        acc += np.einsum("ij,ji->i", s[bc], w_res[:, rc], dtype=np.float64,
                         optimize=True)
        g_acc = np.einsum("ij,ji->i", xg, w_gate[:, rc], dtype=np.float64,
                          optimize=True)
        gate_v = 1.0 / (1.0 + np.exp(-g_acc))
        ns_fix = (0.9 * s[bc, rc].astype(np.float64)
                  + 0.1 * np.tanh(acc)) * gate_v
        ns_fix32 = ns_fix.astype(np.float32)
        new_state[bc, rc] = ns_fix32
        output[bc, rc] = (ns_fix32 > 0.5).astype(np.float32)
    return output, new_state
